# revision 44
# baseline (speedup 1.0000x reference)
"""AttentionBlock (GroupNorm + QKV 1x1conv + full attention + proj + residual)
for Trainium2, data-parallel over (batch, query-half) across 8 NeuronCores.

Self-contained: hardcodes shapes B=4, C=256, H=W=64 from the problem spec.

Per-core plan (core c: batch b=c//2, query-half h=c%2):
  - Host rolls x[b]'s pixel axis so this core's 2048 queries are columns
    0:2048 (attention is permutation-invariant over keys, and GroupNorm
    stats are permutation-invariant, so a single SPMD program serves all
    cores).
  - GroupNorm -> h8 (fp8e4, per-channel affine folded into one activation).
  - All large matmuls run in fp8e4 with perf_mode=DoubleRow (2 fp8 MACs
    per PE per cycle, K=256 contracted per pass):
      K/Q = w.T @ h (biases added on the DVE during PSUM->SBUF eviction),
      VT[m, c] built directly transposed, S^T = K.T Q, PV, and the softmax
      denominator D = ones.T @ exp(S^T/16 - 2).
  - softmax: unnormalized exp on ScalarE (fp8 output); 1/D via
    reciprocal_approx_fast (DVE), broadcast across partitions with a
    1-row f32r matmul; normalization folded AFTER the bf16 projection
    (proj(att)*diag(1/D) == proj(att*diag(1/D))), which takes the slow
    reciprocal off the PE critical path.
  - out = x + proj(PV)/D + pb2, with pb2 = w_proj@bv + b_proj.

PSUM: one 4-slot ring [128,4,512] shared by S^T / K / Q / V / proj /
broadcast, a persistent [128,2,512] PV accumulator, and a double-buffered
[1,512] denominator accumulator. Per-chunk tails (proj, broadcast, final
residual add) are emitted 1-4 iterations into the NEXT chunk's S^T stream
so the PE never stalls waiting on the softmax denominator.

The toolchain's walrus build accepts only one sync-wait per instruction,
so a post-pass splits multi-wait instructions into NoOp chains.
"""

import sys

if "/opt/trn_rl_repo" not in sys.path:
    sys.path.insert(0, "/opt/trn_rl_repo")

import numpy as np

import concourse.bass as bass
import concourse.mybir as mybir
import concourse.tile as tile
from concourse.bass_utils import run_bass_kernel_spmd

F32 = mybir.dt.float32
F32R = mybir.dt.float32r
F8 = mybir.dt.float8e4
BF16 = mybir.dt.bfloat16
AF = mybir.ActivationFunctionType
ALU = mybir.AluOpType
DR = mybir.MatmulPerfMode.DoubleRow

B, C, HH, WW = 4, 256, 64, 64
N = HH * WW          # 4096 pixels
G = 8                # groups
QH = N // 2          # queries per core
NCORES = 8
EPS = 1e-5
INV_CNT = 1.0 / (32 * N)   # 1 / elements per group
SM_SCALE = 1.0 / 16.0      # 1/sqrt(C)
EXP_BIAS = -4.0            # exp(s - 4): keeps fp8 exps inside e4m3 range
                           # (max observed logit ~8.0 + fp8 q/k noise; the
                           # shift cancels exactly in the softmax ratio)

CT = C // 128        # 2 channel tiles
MB = N // 128        # 32 key blocks
NCH = QH // 512      # 4 query chunks per core
NUP = MB // 2        # 16 key-block pairs


# ---------------------------------------------------------------------------
# walrus in this env allows only ONE sync-wait command per instruction.
# Split multi-wait instructions into single-wait NoOps (same engine, so
# in-order execution preserves the blocking semantics exactly).
_ws_counter = [0]


def _split_block(b):
    new = []
    changed = False
    for ins in b.instructions:
        si = ins.sync_info
        if si is not None and si.on_wait and len(si.on_wait) > 1:
            waits = list(si.on_wait)
            for w in waits[:-1]:
                _ws_counter[0] += 1
                new.append(mybir.InstNoOp(
                    name=f"I-waitsplit-{_ws_counter[0]}",
                    engine=ins.engine,
                    sync_info=mybir.SyncInfo(on_wait=[w], on_update=[]),
                ))
            ins.sync_info = mybir.SyncInfo(
                on_wait=[waits[-1]], on_update=list(si.on_update or []))
            changed = True
        new.append(ins)
    if changed:
        b.instructions[:] = new
    for sub in getattr(b, "blocks", []) or []:
        _split_block(sub)


def split_multi_waits(nc):
    for b in nc.main_func.blocks:
        _split_block(b)
    return nc


# ---------------------------------------------------------------------------
import os as _os
_DEBUG_STOP = int(_os.environ.get("KERNEL_DEBUG_STOP", "99"))
_DEBUG_TAIL = int(_os.environ.get("KERNEL_DEBUG_TAIL", "99"))


class _StopBuild(Exception):
    pass


def build():
    nc = bass.Bass()

    X = nc.dram_tensor("x", [C, N], F32, kind="ExternalInput")
    WQ8 = nc.dram_tensor("wq8", [128, CT, C], F8, kind="ExternalInput")
    WK8 = nc.dram_tensor("wk8", [128, CT, C], F8, kind="ExternalInput")
    WV8 = nc.dram_tensor("wv8", [128, CT, C], F8, kind="ExternalInput")
    WPT16 = nc.dram_tensor("wpt16", [C, C], BF16, kind="ExternalInput")
    BQ = nc.dram_tensor("bq2", [128, CT], F32, kind="ExternalInput")
    BK = nc.dram_tensor("bk2", [128, CT], F32, kind="ExternalInput")
    PB2 = nc.dram_tensor("pb2", [128, CT], F32, kind="ExternalInput")
    GAMMA = nc.dram_tensor("gamma2", [128, CT], F32, kind="ExternalInput")
    BETA = nc.dram_tensor("beta2", [128, CT], F32, kind="ExternalInput")
    GMASK = nc.dram_tensor("gmask", [128, 4], F32R, kind="ExternalInput")
    GBCAST = nc.dram_tensor("gbcast", [4, 128], F32R, kind="ExternalInput")
    ONES8 = nc.dram_tensor("ones8", [128, CT, 16], F8, kind="ExternalInput")
    ONESROW = nc.dram_tensor("ones1x128", [1, 128], F32R, kind="ExternalInput")
    EPSC = nc.dram_tensor("epsc", [128, 1], F32, kind="ExternalInput")
    EXPB = nc.dram_tensor("expb", [128, 1], F32, kind="ExternalInput")
    OUT = nc.dram_tensor("out", [C, QH], F32, kind="ExternalOutput")
    dbg = {}
    if _os.environ.get("KERNEL_DEBUG_DUMP"):
        dbg["h8"] = nc.dram_tensor("dbg_h8", [128, CT, N], F8,
                                   kind="ExternalOutput")
        dbg["k8"] = nc.dram_tensor("dbg_k8", [128, CT, N], F8,
                                   kind="ExternalOutput")
        dbg["q8"] = nc.dram_tensor("dbg_q8", [128, CT, QH], F8,
                                   kind="ExternalOutput")
        dbg["vt8"] = nc.dram_tensor("dbg_vt8", [128, MB, C], F8,
                                    kind="ExternalOutput")
        dbg["att"] = nc.dram_tensor("dbg_att", [128, 2, 512], F32,
                                    kind="ExternalOutput")
        dbg["drec"] = nc.dram_tensor("dbg_drec", [1, 512], F32R,
                                     kind="ExternalOutput")
        dbg["exps"] = nc.dram_tensor("dbg_exps", [128, 2, 512], F8,
                                     kind="ExternalOutput")

    handles = dict(X=X, WQ8=WQ8, WK8=WK8, WV8=WV8, WPT16=WPT16, BQ=BQ,
                   BK=BK, PB2=PB2, GAMMA=GAMMA, BETA=BETA, GMASK=GMASK,
                   GBCAST=GBCAST, ONES8=ONES8, ONESROW=ONESROW, EPSC=EPSC,
                   EXPB=EXPB, OUT=OUT)
    try:
        _build_body(nc, dbg, handles)
    except _StopBuild:
        pass
    split_multi_waits(nc)
    return nc


def _build_body(nc, dbg, handles):
    (X, WQ8, WK8, WV8, WPT16, BQ, BK, PB2, GAMMA, BETA, GMASK, GBCAST,
     ONES8, ONESROW, EPSC, EXPB, OUT) = (
        handles[n] for n in (
            "X", "WQ8", "WK8", "WV8", "WPT16", "BQ", "BK", "PB2",
            "GAMMA", "BETA", "GMASK", "GBCAST", "ONES8", "ONESROW",
            "EPSC", "EXPB", "OUT"))
    with tile.TileContext(nc) as tc, nc.allow_low_precision(
            reason="fp8/bf16 attention with fp32 accumulation"):
        with tc.tile_pool(name="big", bufs=1) as big, \
             tc.tile_pool(name="small", bufs=1) as small, \
             tc.tile_pool(name="expp", bufs=3) as expp, \
             tc.tile_pool(name="attp", bufs=2) as attp, \
             tc.tile_pool(name="dbp", bufs=2) as dbp, \
             tc.tile_pool(name="outp", bufs=4) as outp, \
             tc.tile_pool(name="ps_ring", bufs=1, space="PSUM") as ps_ring, \
             tc.tile_pool(name="ps_att", bufs=1, space="PSUM") as ps_attp, \
             tc.tile_pool(name="ps_p", bufs=1, space="PSUM") as ps_pp, \
             tc.tile_pool(name="ps_d", bufs=1, space="PSUM") as ps_dp:

            # ---------------------------------------------- PSUM ring
            ring = ps_ring.tile([128, 4, 512], F32, tag="ring")
            _rctr = [0]

            def rslot():
                i = _rctr[0] % 4
                _rctr[0] += 1
                return ring[:, i, :]

            def rpair():
                if _rctr[0] % 2:
                    _rctr[0] += 1
                i = _rctr[0] % 4
                _rctr[0] += 2
                return ring[:, i:i + 2, :]

            # ------------------------------------------------ load x + weights
            x_sb = [big.tile([128, N], F32, tag=f"x{t}", name=f"x{t}")
                    for t in range(CT)]
            for t in range(CT):
                nc.sync.dma_start(x_sb[t][:], X[t * 128:(t + 1) * 128, :])

            wq8 = small.tile([128, CT, C], F8, tag="wq8")
            wk8 = small.tile([128, CT, C], F8, tag="wk8")
            wv8 = small.tile([128, CT, C], F8, tag="wv8")
            for t_, d_ in ((wq8, WQ8), (wk8, WK8), (wv8, WV8)):
                nc.sync.dma_start(t_[:], d_[:])
            wp16 = [small.tile([128, C], BF16, tag=f"wp{t}", name=f"wp{t}")
                    for t in range(CT)]
            for t in range(CT):
                nc.sync.dma_start(wp16[t][:], WPT16[t * 128:(t + 1) * 128, :])

            bq = small.tile([128, CT], F32, tag="bq")
            bk = small.tile([128, CT], F32, tag="bk")
            pb2 = small.tile([128, CT], F32, tag="pb2")
            gamma = small.tile([128, CT], F32, tag="gamma")
            beta = small.tile([128, CT], F32, tag="beta")
            for t_, d_ in ((bq, BQ), (bk, BK), (pb2, PB2), (gamma, GAMMA),
                           (beta, BETA)):
                nc.sync.dma_start(t_[:], d_[:])
            gmask = small.tile([128, 4], F32R, tag="gmask")
            nc.sync.dma_start(gmask[:], GMASK[:])
            gbcast = small.tile([4, 128], F32R, tag="gbcast")
            nc.sync.dma_start(gbcast[:], GBCAST[:])
            ones8 = small.tile([128, CT, 16], F8, tag="ones8")
            nc.sync.dma_start(ones8[:], ONES8[:])
            onesrow = small.tile([1, 128], F32R, tag="onesrow")
            nc.sync.dma_start(onesrow[:], ONESROW[:])
            epsc = small.tile([128, 1], F32, tag="epsc")
            nc.sync.dma_start(epsc[:], EPSC[:])
            expb = small.tile([128, 1], F32, tag="expb")
            nc.sync.dma_start(expb[:], EXPB[:])

            # ------------------------------------------------ GroupNorm stats
            h8 = big.tile([128, CT, N], F8, tag="h8")
            k8 = big.tile([128, CT, N], F8, tag="k8")
            q8 = big.tile([128, CT, QH], F8, tag="q8")
            vt8 = big.tile([128, MB, C], F8, tag="vt8")

            stats = [small.tile([128, 2], F32R, tag=f"st{t}", name=f"st{t}")
                     for t in range(CT)]
            for t in range(CT):
                nc.vector.tensor_reduce(
                    stats[t][:, 0:1], x_sb[t][:], axis=mybir.AxisListType.X,
                    op=ALU.add)
                # Square output is pure scratch (k8 is overwritten later);
                # the f32 accumulated row-sum is what we keep.
                nc.scalar.activation(
                    k8[:, t, :], x_sb[t][:], AF.Square,
                    accum_out=stats[t][:, 1:2])

            a_c = []
            d_c = []
            for t in range(CT):
                ps_g = rslot()
                nc.tensor.matmul(ps_g[0:4, 0:2], gmask[:], stats[t][:],
                                 start=True, stop=True)
                gstats = small.tile([4, 2], F32R, tag=f"gst{t}", name=f"gst{t}")
                nc.vector.tensor_copy(gstats[:], ps_g[0:4, 0:2])
                ps_bc = rslot()
                nc.tensor.matmul(ps_bc[:, 0:2], gbcast[:], gstats[:],
                                 start=True, stop=True)
                mean = small.tile([128, 1], F32, tag=f"mean{t}")
                ex2 = small.tile([128, 1], F32, tag=f"ex2{t}")
                nc.scalar.mul(mean[:], ps_bc[:, 0:1], INV_CNT)
                nc.scalar.mul(ex2[:], ps_bc[:, 1:2], INV_CNT)
                var = small.tile([128, 1], F32, tag=f"var{t}")
                # var' = mean^2 - ex2; rstd = exp(-0.5*ln(eps - var'))
                nc.vector.scalar_tensor_tensor(
                    var[:], mean[:], mean[:], ex2[:],
                    op0=ALU.mult, op1=ALU.subtract)
                lnv = small.tile([128, 1], F32, tag=f"lnv{t}")
                nc.scalar.activation(lnv[:], var[:], AF.Ln,
                                     bias=epsc[:], scale=-1.0)
                rstd = small.tile([128, 1], F32, tag=f"rstd{t}")
                nc.scalar.activation(rstd[:], lnv[:], AF.Exp,
                                     bias=0.0, scale=-0.5)
                a_t = small.tile([128, 1], F32, tag=f"a{t}")
                nc.vector.tensor_mul(a_t[:], rstd[:], gamma[:, t:t + 1])
                d_t = small.tile([128, 1], F32, tag=f"d{t}")
                tmp = small.tile([128, 1], F32, tag=f"tmp{t}")
                nc.vector.tensor_mul(tmp[:], mean[:], a_t[:])
                nc.vector.tensor_sub(d_t[:], beta[:, t:t + 1], tmp[:])
                a_c.append(a_t)
                d_c.append(d_t)

            for t in range(CT):
                nc.scalar.activation(h8[:, t, :], x_sb[t][:], AF.Identity,
                                     bias=d_c[t][:], scale=a_c[t][:])

            # ------------------------------------------------ K / Q (fp8 DR)
            if _DEBUG_STOP < 1:
                raise _StopBuild
            # K[ot*128+m, n] = sum_c wk[ot*128+m, c] h[c, n]; PSUM->SBUF
            # bias-add evictions all run on the DVE: one engine draining
            # back-to-back paces the ring better than alternating engines
            # (each handoff costs two sem round-trips and cools the PE).
            def evict_add(dst, ps, bias_col):
                nc.vector.tensor_scalar_add(dst, ps, bias_col)

            for ot in range(CT):
                for cp in range(N // 1024):
                    ps = rpair()
                    for j in range(2):
                        ch = 2 * cp + j
                        nc.tensor.matmul(
                            ps[:, j, :],
                            wk8[:, :, ot * 128:(ot + 1) * 128],
                            h8[:, :, ch * 512:(ch + 1) * 512],
                            start=True, stop=True, perf_mode=DR)
                    evict_add(k8[:, ot, cp * 1024:(cp + 1) * 1024], ps,
                              bk[:, ot:ot + 1])
            for ot in range(CT):
                for cp in range(QH // 1024):
                    ps = rpair()
                    for j in range(2):
                        ch = 2 * cp + j
                        nc.tensor.matmul(
                            ps[:, j, :],
                            wq8[:, :, ot * 128:(ot + 1) * 128],
                            h8[:, :, ch * 512:(ch + 1) * 512],
                            start=True, stop=True, perf_mode=DR)
                    evict_add(q8[:, ot, cp * 1024:(cp + 1) * 1024], ps,
                              bq[:, ot:ot + 1])

            if _DEBUG_STOP < 2:
                raise _StopBuild
            # ------------------------------------------------ V^T (fp8 DR)
            # VT[u*128+m, vc] = sum_c h[c, u*128+m] wv[vc, c]   (bv -> pb2)
            for up in range(NUP):
                ps = rslot()
                for j in range(2):
                    u = 2 * up + j
                    nc.tensor.matmul(
                        ps[:, j * 256:(j + 1) * 256],
                        h8[:, :, u * 128:(u + 1) * 128],
                        wv8[:],
                        start=True, stop=True, perf_mode=DR)
                nc.vector.tensor_copy(vt8[:, 2 * up:2 * up + 2, :], ps)

            # ------------------------------------------------ attention
            if _DEBUG_STOP < 3:
                raise _StopBuild
            resid = x_sb  # x_sb[ot][:, 0:QH] is this core's residual slice

            # tail state carried from chunk ch-1 into chunk ch's up-loop
            tail = [None]

            def emit_tail(step, ch_prev):
                """step 0: bcast 1/D; 1: proj+final ot0; 2: proj+final ot1.

                The shared ps_p bank serializes bcast -> proj0 -> proj1, but
                each writer only waits on the previous tenant's single DVE
                read, and S^T work is interleaved between steps, so the PE
                stream stays fed."""
                st = tail[0]
                if st is None or st["ch"] != ch_prev:
                    return
                ch = st["ch"]
                qs = slice(ch * 512, (ch + 1) * 512)
                if step == 0:
                    ps_db = ps_pp.tile([128, 512], F32, tag="pp")
                    nc.tensor.matmul(ps_db[:], onesrow[:], st["drec"][:],
                                     start=True, stop=True)
                    db_sb = dbp.tile([128, 512], F32, tag="db")
                    nc.vector.tensor_copy(db_sb[:], ps_db[:])
                    st["db"] = db_sb
                else:
                    ot = step - 1
                    ps_p = ps_pp.tile([128, 512], F32, tag="pp")
                    for ctk in range(CT):
                        nc.tensor.matmul(
                            ps_p[:],
                            wp16[ctk][:, ot * 128:(ot + 1) * 128],
                            st["att"][:, ctk, :],
                            start=(ctk == 0), stop=(ctk == CT - 1))
                    t1 = outp.tile([128, 512], F32, tag="t1")
                    nc.vector.tensor_mul(t1[:], ps_p[:], st["db"][:])
                    o_t = outp.tile([128, 512], F32, tag="o")
                    nc.vector.scalar_tensor_tensor(
                        o_t[:], t1[:], pb2[:, ot:ot + 1],
                        resid[ot][:, qs],
                        op0=ALU.add, op1=ALU.add)
                    nc.sync.dma_start(OUT[ot * 128:(ot + 1) * 128, qs],
                                      o_t[:])
                    if step == 2:
                        tail[0] = None

            # Software-pipelined PE stream: S^T for iteration g+1 is emitted
            # BEFORE PV/D of iteration g, so the PE computes the next scores
            # block while ScalarE runs exp(g) — the exp handoff latency never
            # stalls the (in-order) PE queue.
            def emit_st(g):
                ch, up = divmod(g, NUP)
                qs = slice(ch * 512, (ch + 1) * 512)
                ps_st = rpair()
                for j in range(2):
                    u = 2 * up + j
                    nc.tensor.matmul(
                        ps_st[:, j, :],
                        k8[:, :, u * 128:(u + 1) * 128],
                        q8[:, :, qs],
                        start=True, stop=True, perf_mode=DR)
                return ps_st

            ps_att = None
            ps_d = None
            st_next = emit_st(0) if NCH else None
            for g in range(NCH * NUP):
                ch, up = divmod(g, NUP)
                if up == 0:
                    ps_att = ps_attp.tile([128, 2, 512], F32, tag="psatt")
                    ps_d = ps_dp.tile([1, 512], F32, tag="psd")
                ps_st = st_next
                exps = expp.tile([128, 2, 512], F8, tag="exps")
                nc.scalar.activation(exps[:], ps_st, AF.Exp,
                                     bias=expb[:], scale=SM_SCALE)
                if dbg and g == 0:
                    nc.sync.dma_start(dbg["exps"][:], exps[:])
                if g + 1 < NCH * NUP:
                    st_next = emit_st(g + 1)
                for ct in range(CT):
                    nc.tensor.matmul(
                        ps_att[:, ct, :],
                        vt8[:, 2 * up:2 * up + 2,
                            ct * 128:(ct + 1) * 128],
                        exps[:],
                        start=(up == 0), stop=(up == NUP - 1),
                        perf_mode=DR)
                nc.tensor.matmul(
                    ps_d[:], ones8[:, :, 0:1], exps[:],
                    start=(up == 0), stop=(up == NUP - 1),
                    perf_mode=DR)
                if _DEBUG_STOP >= 5 and up in (3, 5, 7):
                    emit_tail((up - 3) // 2, ch - 1)

                if up != NUP - 1 or _DEBUG_TAIL < 1:
                    continue
                # end of chunk: immediately evict this chunk's accumulators
                att16 = attp.tile([128, 2, 512], BF16, tag="att")
                nc.vector.tensor_copy(att16[:], ps_att[:])
                drec = dbp.tile([1, 512], F32R, tag="drec")
                nc.vector.reciprocal(drec[:], ps_d[:])
                tail[0] = {"ch": ch, "att": att16, "drec": drec}
                if dbg and ch == 0:
                    att32 = attp.tile([128, 2, 512], F32, tag="att32")
                    nc.vector.tensor_copy(att32[:], ps_att[:])
                    nc.sync.dma_start(dbg["att"][:], att32[:])
                    nc.sync.dma_start(dbg["drec"][:], drec[:])
                if _DEBUG_STOP == 4 or (_DEBUG_STOP == 3 and ch == 0):
                    for step in range(min(3, _DEBUG_TAIL - 1)):
                        emit_tail(step, ch)
                if _DEBUG_STOP == 3 and ch == 0:
                    break

            if _DEBUG_STOP >= 5:
                for step in range(3):
                    emit_tail(step, NCH - 1)

            if dbg:
                for name, src in (("h8", h8), ("k8", k8), ("q8", q8),
                                  ("vt8", vt8)):
                    nc.sync.dma_start(dbg[name][:], src[:])


_NC_CACHE = None


def _get_nc():
    global _NC_CACHE
    if _NC_CACHE is None:
        _NC_CACHE = build()
    return _NC_CACHE


def make_in_maps(x, gamma, beta, w_qkv, b_qkv, w_proj, b_proj):
    np_f8 = mybir.dt.np(F8)
    np_bf16 = mybir.dt.np(BF16)
    x = np.asarray(x, np.float32).reshape(B, C, N)
    gamma = np.asarray(gamma, np.float32)
    beta = np.asarray(beta, np.float32)
    w_qkv = np.asarray(w_qkv, np.float32)
    b_qkv = np.asarray(b_qkv, np.float32)
    w_proj = np.asarray(w_proj, np.float32)
    b_proj = np.asarray(b_proj, np.float32)

    wq, wk, wv = w_qkv[0:C], w_qkv[C:2 * C], w_qkv[2 * C:3 * C]
    bq, bk, bv = b_qkv[0:C], b_qkv[C:2 * C], b_qkv[2 * C:3 * C]
    pb2 = (w_proj @ bv + b_proj).astype(np.float32)

    def col2(v):
        return np.ascontiguousarray(v.reshape(CT, 128).T, dtype=np.float32)

    def dr_w(w):
        # [128, CT, C]: out[p, t, o] = w[o, t*128 + p]
        return np.ascontiguousarray(
            w.T.reshape(CT, 128, C).transpose(1, 0, 2)).astype(np_f8)

    gmask = np.zeros((128, 4), np.float32)
    for p in range(128):
        gmask[p, p // 32] = 1.0
    gbcast = np.ascontiguousarray(gmask.T)

    common = {
        "wq8": dr_w(wq),
        "wk8": dr_w(wk),
        "wv8": dr_w(wv),
        "wpt16": np.ascontiguousarray(w_proj.T).astype(np_bf16),
        "bq2": col2(bq),
        "bk2": col2(bk),
        "pb2": col2(pb2),
        "gamma2": col2(gamma),
        "beta2": col2(beta),
        "gmask": gmask,
        "gbcast": gbcast,
        "ones8": np.ones((128, CT, 16), np_f8),
        "ones1x128": np.ones((1, 128), np.float32),
        "epsc": np.full((128, 1), EPS, np.float32),
        "expb": np.full((128, 1), EXP_BIAS, np.float32),
    }

    in_maps = []
    for core in range(NCORES):
        b, half = core // 2, core % 2
        qoff = half * QH
        xc = np.concatenate([x[b][:, qoff:], x[b][:, :qoff]], axis=1)
        m = dict(common)
        m["x"] = np.ascontiguousarray(xc)
        in_maps.append(m)
    return in_maps


def gather_out(results):
    out = np.empty((B, C, N), np.float32)
    for core in range(NCORES):
        b, half = core // 2, core % 2
        qoff = half * QH
        out[b][:, qoff:qoff + QH] = results[core]["out"]
    return out.reshape(B, C, HH, WW)


def kernel(x, gamma, beta, w_qkv, b_qkv, w_proj, b_proj, **run_kwargs):
    nc = _get_nc()
    in_maps = make_in_maps(x, gamma, beta, w_qkv, b_qkv, w_proj, b_proj)
    res = run_bass_kernel_spmd(nc, in_maps, core_ids=list(range(NCORES)),
                               **run_kwargs)
    out = gather_out(res.results)
    kernel.last_results = res
    return out


# revision 45
# speedup vs baseline: 1.0333x; 1.0333x over previous
"""AttentionBlock (GroupNorm + QKV 1x1conv + full attention + proj + residual)
for Trainium2, data-parallel over (batch, query-half) across 8 NeuronCores.

Self-contained: hardcodes shapes B=4, C=256, H=W=64 from the problem spec.

Per-core plan (core c: batch b=c//2, query-half h=c%2):
  - Host rolls x[b]'s pixel axis so this core's 2048 queries are columns
    0:2048 (attention is permutation-invariant over keys, and GroupNorm
    stats are permutation-invariant, so a single SPMD program serves all
    cores).
  - GroupNorm -> h8 (fp8e4, per-channel affine folded into one activation).
  - All large matmuls run in fp8e4 with perf_mode=DoubleRow (2 fp8 MACs
    per PE per cycle, K=256 contracted per pass):
      K/Q = w.T @ h (biases added on the DVE during PSUM->SBUF eviction),
      VT[m, c] built directly transposed, S^T = K.T Q, PV, and the softmax
      denominator D = ones.T @ exp(S^T/16 - 2).
  - softmax: unnormalized exp on ScalarE (fp8 output); 1/D via
    reciprocal_approx_fast (DVE), broadcast across partitions with a
    1-row f32r matmul; normalization folded AFTER the bf16 projection
    (proj(att)*diag(1/D) == proj(att*diag(1/D))), which takes the slow
    reciprocal off the PE critical path.
  - out = x + proj(PV)/D + pb2, with pb2 = w_proj@bv + b_proj.

PSUM: one 4-slot ring [128,4,512] shared by S^T / K / Q / V / proj /
broadcast, a persistent [128,2,512] PV accumulator, and a double-buffered
[1,512] denominator accumulator. Per-chunk tails (proj, broadcast, final
residual add) are emitted 1-4 iterations into the NEXT chunk's S^T stream
so the PE never stalls waiting on the softmax denominator.

The toolchain's walrus build accepts only one sync-wait per instruction,
so a post-pass splits multi-wait instructions into NoOp chains.
"""

import sys

if "/opt/trn_rl_repo" not in sys.path:
    sys.path.insert(0, "/opt/trn_rl_repo")

import numpy as np

import concourse.bass as bass
import concourse.mybir as mybir
import concourse.tile as tile
from concourse.bass_utils import run_bass_kernel_spmd

F32 = mybir.dt.float32
F32R = mybir.dt.float32r
F8 = mybir.dt.float8e4
BF16 = mybir.dt.bfloat16
AF = mybir.ActivationFunctionType
ALU = mybir.AluOpType
DR = mybir.MatmulPerfMode.DoubleRow

B, C, HH, WW = 4, 256, 64, 64
N = HH * WW          # 4096 pixels
G = 8                # groups
QH = N // 2          # queries per core
NCORES = 8
EPS = 1e-5
INV_CNT = 1.0 / (32 * N)   # 1 / elements per group
SM_SCALE = 1.0 / 16.0      # 1/sqrt(C)
EXP_BIAS = -3.0            # exp(s - 3): keeps fp8 exps inside e4m3 range
                           # (max observed logit ~8.0 + fp8 q/k noise; the
                           # shift cancels exactly in the softmax ratio).
                           # More negative wastes e4m3 range on subnormals,
                           # which measurably slows the fp8 datapath.

CT = C // 128        # 2 channel tiles
MB = N // 128        # 32 key blocks
NCH = QH // 512      # 4 query chunks per core
NUP = MB // 2        # 16 key-block pairs


# ---------------------------------------------------------------------------
# walrus in this env allows only ONE sync-wait command per instruction.
# Split multi-wait instructions into single-wait NoOps (same engine, so
# in-order execution preserves the blocking semantics exactly).
_ws_counter = [0]


def _split_block(b):
    new = []
    changed = False
    for ins in b.instructions:
        si = ins.sync_info
        if si is not None and si.on_wait and len(si.on_wait) > 1:
            waits = list(si.on_wait)
            for w in waits[:-1]:
                _ws_counter[0] += 1
                new.append(mybir.InstNoOp(
                    name=f"I-waitsplit-{_ws_counter[0]}",
                    engine=ins.engine,
                    sync_info=mybir.SyncInfo(on_wait=[w], on_update=[]),
                ))
            ins.sync_info = mybir.SyncInfo(
                on_wait=[waits[-1]], on_update=list(si.on_update or []))
            changed = True
        new.append(ins)
    if changed:
        b.instructions[:] = new
    for sub in getattr(b, "blocks", []) or []:
        _split_block(sub)


def split_multi_waits(nc):
    for b in nc.main_func.blocks:
        _split_block(b)
    return nc


# ---------------------------------------------------------------------------
import os as _os
_DEBUG_STOP = int(_os.environ.get("KERNEL_DEBUG_STOP", "99"))
_DEBUG_TAIL = int(_os.environ.get("KERNEL_DEBUG_TAIL", "99"))


class _StopBuild(Exception):
    pass


def build():
    nc = bass.Bass()

    X = nc.dram_tensor("x", [C, N], F32, kind="ExternalInput")
    WQ8 = nc.dram_tensor("wq8", [128, CT, C], F8, kind="ExternalInput")
    WK8 = nc.dram_tensor("wk8", [128, CT, C], F8, kind="ExternalInput")
    WV8 = nc.dram_tensor("wv8", [128, CT, C], F8, kind="ExternalInput")
    WPT16 = nc.dram_tensor("wpt16", [C, C], BF16, kind="ExternalInput")
    BQ = nc.dram_tensor("bq2", [128, CT], F32, kind="ExternalInput")
    BK = nc.dram_tensor("bk2", [128, CT], F32, kind="ExternalInput")
    PB2 = nc.dram_tensor("pb2", [128, CT], F32, kind="ExternalInput")
    GAMMA = nc.dram_tensor("gamma2", [128, CT], F32, kind="ExternalInput")
    BETA = nc.dram_tensor("beta2", [128, CT], F32, kind="ExternalInput")
    GMASK = nc.dram_tensor("gmask", [128, 4], F32R, kind="ExternalInput")
    GBCAST = nc.dram_tensor("gbcast", [4, 128], F32R, kind="ExternalInput")
    ONES8 = nc.dram_tensor("ones8", [128, CT, 16], F8, kind="ExternalInput")
    ONESROW = nc.dram_tensor("ones1x128", [1, 128], F32R, kind="ExternalInput")
    EPSC = nc.dram_tensor("epsc", [128, 1], F32, kind="ExternalInput")
    EXPB = nc.dram_tensor("expb", [128, 1], F32, kind="ExternalInput")
    OUT = nc.dram_tensor("out", [C, QH], F32, kind="ExternalOutput")
    dbg = {}
    if _os.environ.get("KERNEL_DEBUG_DUMP"):
        dbg["h8"] = nc.dram_tensor("dbg_h8", [128, CT, N], F8,
                                   kind="ExternalOutput")
        dbg["k8"] = nc.dram_tensor("dbg_k8", [128, CT, N], F8,
                                   kind="ExternalOutput")
        dbg["q8"] = nc.dram_tensor("dbg_q8", [128, CT, QH], F8,
                                   kind="ExternalOutput")
        dbg["vt8"] = nc.dram_tensor("dbg_vt8", [128, MB, C], F8,
                                    kind="ExternalOutput")
        dbg["att"] = nc.dram_tensor("dbg_att", [128, 2, 512], F32,
                                    kind="ExternalOutput")
        dbg["drec"] = nc.dram_tensor("dbg_drec", [1, 512], F32R,
                                     kind="ExternalOutput")
        dbg["exps"] = nc.dram_tensor("dbg_exps", [128, 2, 512], F8,
                                     kind="ExternalOutput")

    handles = dict(X=X, WQ8=WQ8, WK8=WK8, WV8=WV8, WPT16=WPT16, BQ=BQ,
                   BK=BK, PB2=PB2, GAMMA=GAMMA, BETA=BETA, GMASK=GMASK,
                   GBCAST=GBCAST, ONES8=ONES8, ONESROW=ONESROW, EPSC=EPSC,
                   EXPB=EXPB, OUT=OUT)
    try:
        _build_body(nc, dbg, handles)
    except _StopBuild:
        pass
    split_multi_waits(nc)
    return nc


def _build_body(nc, dbg, handles):
    (X, WQ8, WK8, WV8, WPT16, BQ, BK, PB2, GAMMA, BETA, GMASK, GBCAST,
     ONES8, ONESROW, EPSC, EXPB, OUT) = (
        handles[n] for n in (
            "X", "WQ8", "WK8", "WV8", "WPT16", "BQ", "BK", "PB2",
            "GAMMA", "BETA", "GMASK", "GBCAST", "ONES8", "ONESROW",
            "EPSC", "EXPB", "OUT"))
    with tile.TileContext(nc) as tc, nc.allow_low_precision(
            reason="fp8/bf16 attention with fp32 accumulation"):
        with tc.tile_pool(name="big", bufs=1) as big, \
             tc.tile_pool(name="small", bufs=1) as small, \
             tc.tile_pool(name="expp", bufs=3) as expp, \
             tc.tile_pool(name="attp", bufs=2) as attp, \
             tc.tile_pool(name="dbp", bufs=2) as dbp, \
             tc.tile_pool(name="outp", bufs=4) as outp, \
             tc.tile_pool(name="ps_ring", bufs=1, space="PSUM") as ps_ring, \
             tc.tile_pool(name="ps_att", bufs=1, space="PSUM") as ps_attp, \
             tc.tile_pool(name="ps_p", bufs=1, space="PSUM") as ps_pp, \
             tc.tile_pool(name="ps_d", bufs=1, space="PSUM") as ps_dp:

            # ---------------------------------------------- PSUM ring
            ring = ps_ring.tile([128, 4, 512], F32, tag="ring")
            _rctr = [0]

            def rslot():
                i = _rctr[0] % 4
                _rctr[0] += 1
                return ring[:, i, :]

            def rpair():
                if _rctr[0] % 2:
                    _rctr[0] += 1
                i = _rctr[0] % 4
                _rctr[0] += 2
                return ring[:, i:i + 2, :]

            # ------------------------------------------------ load x + weights
            x_sb = [big.tile([128, N], F32, tag=f"x{t}", name=f"x{t}")
                    for t in range(CT)]
            for t in range(CT):
                nc.sync.dma_start(x_sb[t][:], X[t * 128:(t + 1) * 128, :])

            wq8 = small.tile([128, CT, C], F8, tag="wq8")
            wk8 = small.tile([128, CT, C], F8, tag="wk8")
            wv8 = small.tile([128, CT, C], F8, tag="wv8")
            for t_, d_ in ((wq8, WQ8), (wk8, WK8), (wv8, WV8)):
                nc.sync.dma_start(t_[:], d_[:])
            wp16 = [small.tile([128, C], BF16, tag=f"wp{t}", name=f"wp{t}")
                    for t in range(CT)]
            for t in range(CT):
                nc.sync.dma_start(wp16[t][:], WPT16[t * 128:(t + 1) * 128, :])

            bq = small.tile([128, CT], F32, tag="bq")
            bk = small.tile([128, CT], F32, tag="bk")
            pb2 = small.tile([128, CT], F32, tag="pb2")
            gamma = small.tile([128, CT], F32, tag="gamma")
            beta = small.tile([128, CT], F32, tag="beta")
            for t_, d_ in ((bq, BQ), (bk, BK), (pb2, PB2), (gamma, GAMMA),
                           (beta, BETA)):
                nc.sync.dma_start(t_[:], d_[:])
            gmask = small.tile([128, 4], F32R, tag="gmask")
            nc.sync.dma_start(gmask[:], GMASK[:])
            gbcast = small.tile([4, 128], F32R, tag="gbcast")
            nc.sync.dma_start(gbcast[:], GBCAST[:])
            ones8 = small.tile([128, CT, 16], F8, tag="ones8")
            nc.sync.dma_start(ones8[:], ONES8[:])
            onesrow = small.tile([1, 128], F32R, tag="onesrow")
            nc.sync.dma_start(onesrow[:], ONESROW[:])
            epsc = small.tile([128, 1], F32, tag="epsc")
            nc.sync.dma_start(epsc[:], EPSC[:])
            expb = small.tile([128, 1], F32, tag="expb")
            nc.sync.dma_start(expb[:], EXPB[:])

            # ------------------------------------------------ GroupNorm stats
            h8 = big.tile([128, CT, N], F8, tag="h8")
            k8 = big.tile([128, CT, N], F8, tag="k8")
            q8 = big.tile([128, CT, QH], F8, tag="q8")
            vt8 = big.tile([128, MB, C], F8, tag="vt8")

            stats = [small.tile([128, 2], F32R, tag=f"st{t}", name=f"st{t}")
                     for t in range(CT)]
            for t in range(CT):
                nc.vector.tensor_reduce(
                    stats[t][:, 0:1], x_sb[t][:], axis=mybir.AxisListType.X,
                    op=ALU.add)
                # Square output is pure scratch (k8 is overwritten later);
                # the f32 accumulated row-sum is what we keep.
                nc.scalar.activation(
                    k8[:, t, :], x_sb[t][:], AF.Square,
                    accum_out=stats[t][:, 1:2])

            a_c = []
            d_c = []
            for t in range(CT):
                ps_g = rslot()
                nc.tensor.matmul(ps_g[0:4, 0:2], gmask[:], stats[t][:],
                                 start=True, stop=True)
                gstats = small.tile([4, 2], F32R, tag=f"gst{t}", name=f"gst{t}")
                nc.vector.tensor_copy(gstats[:], ps_g[0:4, 0:2])
                ps_bc = rslot()
                nc.tensor.matmul(ps_bc[:, 0:2], gbcast[:], gstats[:],
                                 start=True, stop=True)
                mean = small.tile([128, 1], F32, tag=f"mean{t}")
                ex2 = small.tile([128, 1], F32, tag=f"ex2{t}")
                nc.scalar.mul(mean[:], ps_bc[:, 0:1], INV_CNT)
                nc.scalar.mul(ex2[:], ps_bc[:, 1:2], INV_CNT)
                var = small.tile([128, 1], F32, tag=f"var{t}")
                # var' = mean^2 - ex2; rstd = exp(-0.5*ln(eps - var'))
                nc.vector.scalar_tensor_tensor(
                    var[:], mean[:], mean[:], ex2[:],
                    op0=ALU.mult, op1=ALU.subtract)
                lnv = small.tile([128, 1], F32, tag=f"lnv{t}")
                nc.scalar.activation(lnv[:], var[:], AF.Ln,
                                     bias=epsc[:], scale=-1.0)
                rstd = small.tile([128, 1], F32, tag=f"rstd{t}")
                nc.scalar.activation(rstd[:], lnv[:], AF.Exp,
                                     bias=0.0, scale=-0.5)
                a_t = small.tile([128, 1], F32, tag=f"a{t}")
                nc.vector.tensor_mul(a_t[:], rstd[:], gamma[:, t:t + 1])
                d_t = small.tile([128, 1], F32, tag=f"d{t}")
                tmp = small.tile([128, 1], F32, tag=f"tmp{t}")
                nc.vector.tensor_mul(tmp[:], mean[:], a_t[:])
                nc.vector.tensor_sub(d_t[:], beta[:, t:t + 1], tmp[:])
                a_c.append(a_t)
                d_c.append(d_t)

            for t in range(CT):
                nc.scalar.activation(h8[:, t, :], x_sb[t][:], AF.Identity,
                                     bias=d_c[t][:], scale=a_c[t][:])

            # ------------------------------------------------ K / Q (fp8 DR)
            if _DEBUG_STOP < 1:
                raise _StopBuild
            # K[ot*128+m, n] = sum_c wk[ot*128+m, c] h[c, n]; PSUM->SBUF
            # bias-add evictions all run on the DVE: one engine draining
            # back-to-back paces the ring better than alternating engines
            # (each handoff costs two sem round-trips and cools the PE).
            def evict_add(dst, ps, bias_col):
                nc.vector.tensor_scalar_add(dst, ps, bias_col)

            for ot in range(CT):
                for cp in range(N // 1024):
                    ps = rpair()
                    for j in range(2):
                        ch = 2 * cp + j
                        nc.tensor.matmul(
                            ps[:, j, :],
                            wk8[:, :, ot * 128:(ot + 1) * 128],
                            h8[:, :, ch * 512:(ch + 1) * 512],
                            start=True, stop=True, perf_mode=DR)
                    evict_add(k8[:, ot, cp * 1024:(cp + 1) * 1024], ps,
                              bk[:, ot:ot + 1])
            for ot in range(CT):
                for cp in range(QH // 1024):
                    ps = rpair()
                    for j in range(2):
                        ch = 2 * cp + j
                        nc.tensor.matmul(
                            ps[:, j, :],
                            wq8[:, :, ot * 128:(ot + 1) * 128],
                            h8[:, :, ch * 512:(ch + 1) * 512],
                            start=True, stop=True, perf_mode=DR)
                    evict_add(q8[:, ot, cp * 1024:(cp + 1) * 1024], ps,
                              bq[:, ot:ot + 1])

            if _DEBUG_STOP < 2:
                raise _StopBuild
            # ------------------------------------------------ V^T (fp8 DR)
            # VT[u*128+m, vc] = sum_c h[c, u*128+m] wv[vc, c]   (bv -> pb2)
            for up in range(NUP):
                ps = rslot()
                for j in range(2):
                    u = 2 * up + j
                    nc.tensor.matmul(
                        ps[:, j * 256:(j + 1) * 256],
                        h8[:, :, u * 128:(u + 1) * 128],
                        wv8[:],
                        start=True, stop=True, perf_mode=DR)
                nc.vector.tensor_copy(vt8[:, 2 * up:2 * up + 2, :], ps)

            # ------------------------------------------------ attention
            if _DEBUG_STOP < 3:
                raise _StopBuild
            resid = x_sb  # x_sb[ot][:, 0:QH] is this core's residual slice

            # tail state carried from chunk ch-1 into chunk ch's up-loop
            tail = [None]

            def emit_tail(step, ch_prev):
                """step 0: bcast 1/D; 1: proj+final ot0; 2: proj+final ot1.

                The shared ps_p bank serializes bcast -> proj0 -> proj1, but
                each writer only waits on the previous tenant's single DVE
                read, and S^T work is interleaved between steps, so the PE
                stream stays fed."""
                st = tail[0]
                if st is None or st["ch"] != ch_prev:
                    return
                ch = st["ch"]
                qs = slice(ch * 512, (ch + 1) * 512)
                if step == 0:
                    ps_db = ps_pp.tile([128, 512], F32, tag="pp")
                    nc.tensor.matmul(ps_db[:], onesrow[:], st["drec"][:],
                                     start=True, stop=True)
                    db_sb = dbp.tile([128, 512], F32, tag="db")
                    nc.vector.tensor_copy(db_sb[:], ps_db[:])
                    st["db"] = db_sb
                else:
                    ot = step - 1
                    ps_p = ps_pp.tile([128, 512], F32, tag="pp")
                    for ctk in range(CT):
                        nc.tensor.matmul(
                            ps_p[:],
                            wp16[ctk][:, ot * 128:(ot + 1) * 128],
                            st["att"][:, ctk, :],
                            start=(ctk == 0), stop=(ctk == CT - 1))
                    t1 = outp.tile([128, 512], F32, tag="t1")
                    nc.vector.tensor_mul(t1[:], ps_p[:], st["db"][:])
                    o_t = outp.tile([128, 512], F32, tag="o")
                    nc.vector.scalar_tensor_tensor(
                        o_t[:], t1[:], pb2[:, ot:ot + 1],
                        resid[ot][:, qs],
                        op0=ALU.add, op1=ALU.add)
                    nc.sync.dma_start(OUT[ot * 128:(ot + 1) * 128, qs],
                                      o_t[:])
                    if step == 2:
                        tail[0] = None

            # Software-pipelined PE stream: S^T for iteration g+1 is emitted
            # BEFORE PV/D of iteration g, so the PE computes the next scores
            # block while ScalarE runs exp(g) — the exp handoff latency never
            # stalls the (in-order) PE queue.
            def emit_st(g):
                ch, up = divmod(g, NUP)
                qs = slice(ch * 512, (ch + 1) * 512)
                ps_st = rpair()
                for j in range(2):
                    u = 2 * up + j
                    nc.tensor.matmul(
                        ps_st[:, j, :],
                        k8[:, :, u * 128:(u + 1) * 128],
                        q8[:, :, qs],
                        start=True, stop=True, perf_mode=DR)
                return ps_st

            ps_att = None
            ps_d = None
            st_next = emit_st(0) if NCH else None
            for g in range(NCH * NUP):
                ch, up = divmod(g, NUP)
                if up == 0:
                    ps_att = ps_attp.tile([128, 2, 512], F32, tag="psatt")
                    ps_d = ps_dp.tile([1, 512], F32, tag="psd")
                ps_st = st_next
                exps = expp.tile([128, 2, 512], F8, tag="exps")
                nc.scalar.activation(exps[:], ps_st, AF.Exp,
                                     bias=expb[:], scale=SM_SCALE)
                if dbg and g == 0:
                    nc.sync.dma_start(dbg["exps"][:], exps[:])
                if g + 1 < NCH * NUP:
                    st_next = emit_st(g + 1)
                for ct in range(CT):
                    nc.tensor.matmul(
                        ps_att[:, ct, :],
                        vt8[:, 2 * up:2 * up + 2,
                            ct * 128:(ct + 1) * 128],
                        exps[:],
                        start=(up == 0), stop=(up == NUP - 1),
                        perf_mode=DR)
                nc.tensor.matmul(
                    ps_d[:], ones8[:, :, 0:1], exps[:],
                    start=(up == 0), stop=(up == NUP - 1),
                    perf_mode=DR)
                if _DEBUG_STOP >= 5 and up in (3, 5, 7):
                    emit_tail((up - 3) // 2, ch - 1)

                if up != NUP - 1 or _DEBUG_TAIL < 1:
                    continue
                # end of chunk: immediately evict this chunk's accumulators
                att16 = attp.tile([128, 2, 512], BF16, tag="att")
                nc.vector.tensor_copy(att16[:], ps_att[:])
                drec = dbp.tile([1, 512], F32R, tag="drec")
                nc.vector.reciprocal(drec[:], ps_d[:])
                tail[0] = {"ch": ch, "att": att16, "drec": drec}
                if dbg and ch == 0:
                    att32 = attp.tile([128, 2, 512], F32, tag="att32")
                    nc.vector.tensor_copy(att32[:], ps_att[:])
                    nc.sync.dma_start(dbg["att"][:], att32[:])
                    nc.sync.dma_start(dbg["drec"][:], drec[:])
                if _DEBUG_STOP == 4 or (_DEBUG_STOP == 3 and ch == 0):
                    for step in range(min(3, _DEBUG_TAIL - 1)):
                        emit_tail(step, ch)
                if _DEBUG_STOP == 3 and ch == 0:
                    break

            if _DEBUG_STOP >= 5:
                for step in range(3):
                    emit_tail(step, NCH - 1)

            if dbg:
                for name, src in (("h8", h8), ("k8", k8), ("q8", q8),
                                  ("vt8", vt8)):
                    nc.sync.dma_start(dbg[name][:], src[:])


_NC_CACHE = None


def _get_nc():
    global _NC_CACHE
    if _NC_CACHE is None:
        _NC_CACHE = build()
    return _NC_CACHE


def make_in_maps(x, gamma, beta, w_qkv, b_qkv, w_proj, b_proj):
    np_f8 = mybir.dt.np(F8)
    np_bf16 = mybir.dt.np(BF16)
    x = np.asarray(x, np.float32).reshape(B, C, N)
    gamma = np.asarray(gamma, np.float32)
    beta = np.asarray(beta, np.float32)
    w_qkv = np.asarray(w_qkv, np.float32)
    b_qkv = np.asarray(b_qkv, np.float32)
    w_proj = np.asarray(w_proj, np.float32)
    b_proj = np.asarray(b_proj, np.float32)

    wq, wk, wv = w_qkv[0:C], w_qkv[C:2 * C], w_qkv[2 * C:3 * C]
    bq, bk, bv = b_qkv[0:C], b_qkv[C:2 * C], b_qkv[2 * C:3 * C]
    pb2 = (w_proj @ bv + b_proj).astype(np.float32)

    def col2(v):
        return np.ascontiguousarray(v.reshape(CT, 128).T, dtype=np.float32)

    def dr_w(w):
        # [128, CT, C]: out[p, t, o] = w[o, t*128 + p]
        return np.ascontiguousarray(
            w.T.reshape(CT, 128, C).transpose(1, 0, 2)).astype(np_f8)

    gmask = np.zeros((128, 4), np.float32)
    for p in range(128):
        gmask[p, p // 32] = 1.0
    gbcast = np.ascontiguousarray(gmask.T)

    common = {
        "wq8": dr_w(wq),
        "wk8": dr_w(wk),
        "wv8": dr_w(wv),
        "wpt16": np.ascontiguousarray(w_proj.T).astype(np_bf16),
        "bq2": col2(bq),
        "bk2": col2(bk),
        "pb2": col2(pb2),
        "gamma2": col2(gamma),
        "beta2": col2(beta),
        "gmask": gmask,
        "gbcast": gbcast,
        "ones8": np.ones((128, CT, 16), np_f8),
        "ones1x128": np.ones((1, 128), np.float32),
        "epsc": np.full((128, 1), EPS, np.float32),
        "expb": np.full((128, 1), EXP_BIAS, np.float32),
    }

    in_maps = []
    for core in range(NCORES):
        b, half = core // 2, core % 2
        qoff = half * QH
        xc = np.concatenate([x[b][:, qoff:], x[b][:, :qoff]], axis=1)
        m = dict(common)
        m["x"] = np.ascontiguousarray(xc)
        in_maps.append(m)
    return in_maps


def gather_out(results):
    out = np.empty((B, C, N), np.float32)
    for core in range(NCORES):
        b, half = core // 2, core % 2
        qoff = half * QH
        out[b][:, qoff:qoff + QH] = results[core]["out"]
    return out.reshape(B, C, HH, WW)


def kernel(x, gamma, beta, w_qkv, b_qkv, w_proj, b_proj, **run_kwargs):
    nc = _get_nc()
    in_maps = make_in_maps(x, gamma, beta, w_qkv, b_qkv, w_proj, b_proj)
    res = run_bass_kernel_spmd(nc, in_maps, core_ids=list(range(NCORES)),
                               **run_kwargs)
    out = gather_out(res.results)
    kernel.last_results = res
    return out


# revision 47
# speedup vs baseline: 1.0410x; 1.0074x over previous
"""AttentionBlock (GroupNorm + QKV 1x1conv + full attention + proj + residual)
for Trainium2, data-parallel over (batch, query-half) across 8 NeuronCores.

Self-contained: hardcodes shapes B=4, C=256, H=W=64 from the problem spec.

Per-core plan (core c: batch b=c//2, query-half h=c%2):
  - Host rolls x[b]'s pixel axis so this core's 2048 queries are columns
    0:2048 (attention is permutation-invariant over keys, and GroupNorm
    stats are permutation-invariant, so a single SPMD program serves all
    cores).
  - GroupNorm -> h8 (fp8e4, per-channel affine folded into one activation).
  - All large matmuls run in fp8e4 with perf_mode=DoubleRow (2 fp8 MACs
    per PE per cycle, K=256 contracted per pass):
      K/Q = w.T @ h (biases added on the DVE during PSUM->SBUF eviction),
      VT[m, c] built directly transposed, S^T = K.T Q, PV, and the softmax
      denominator D = ones.T @ exp(S^T/16 - 2).
  - softmax: unnormalized exp on ScalarE (fp8 output); 1/D via
    reciprocal_approx_fast (DVE), broadcast across partitions with a
    1-row f32r matmul; normalization folded AFTER the bf16 projection
    (proj(att)*diag(1/D) == proj(att*diag(1/D))), which takes the slow
    reciprocal off the PE critical path.
  - out = x + proj(PV)/D + pb2, with pb2 = w_proj@bv + b_proj.

PSUM: one 4-slot ring [128,4,512] shared by S^T / K / Q / V / proj /
broadcast, a persistent [128,2,512] PV accumulator, and a double-buffered
[1,512] denominator accumulator. Per-chunk tails (proj, broadcast, final
residual add) are emitted 1-4 iterations into the NEXT chunk's S^T stream
so the PE never stalls waiting on the softmax denominator.

The toolchain's walrus build accepts only one sync-wait per instruction,
so a post-pass splits multi-wait instructions into NoOp chains.
"""

import sys

if "/opt/trn_rl_repo" not in sys.path:
    sys.path.insert(0, "/opt/trn_rl_repo")

import numpy as np

import concourse.bass as bass
import concourse.mybir as mybir
import concourse.tile as tile
from concourse.bass_utils import run_bass_kernel_spmd

F32 = mybir.dt.float32
F32R = mybir.dt.float32r
F8 = mybir.dt.float8e4
BF16 = mybir.dt.bfloat16
AF = mybir.ActivationFunctionType
ALU = mybir.AluOpType
DR = mybir.MatmulPerfMode.DoubleRow

B, C, HH, WW = 4, 256, 64, 64
N = HH * WW          # 4096 pixels
G = 8                # groups
QH = N // 2          # queries per core
NCORES = 8
EPS = 1e-5
INV_CNT = 1.0 / (32 * N)   # 1 / elements per group
SM_SCALE = 1.0 / 16.0      # 1/sqrt(C)
EXP_BIAS = -3.0            # exp(s - 3): keeps fp8 exps inside e4m3 range
                           # (max observed logit ~8.0 + fp8 q/k noise; the
                           # shift cancels exactly in the softmax ratio).
                           # More negative wastes e4m3 range on subnormals,
                           # which measurably slows the fp8 datapath.

CT = C // 128        # 2 channel tiles
MB = N // 128        # 32 key blocks
NCH = QH // 512      # 4 query chunks per core
NUP = MB // 2        # 16 key-block pairs


# ---------------------------------------------------------------------------
# walrus in this env allows only ONE sync-wait command per instruction.
# Split multi-wait instructions into single-wait NoOps (same engine, so
# in-order execution preserves the blocking semantics exactly).
_ws_counter = [0]


def _split_block(b):
    new = []
    changed = False
    for ins in b.instructions:
        si = ins.sync_info
        if si is not None and si.on_wait and len(si.on_wait) > 1:
            waits = list(si.on_wait)
            for w in waits[:-1]:
                _ws_counter[0] += 1
                new.append(mybir.InstNoOp(
                    name=f"I-waitsplit-{_ws_counter[0]}",
                    engine=ins.engine,
                    sync_info=mybir.SyncInfo(on_wait=[w], on_update=[]),
                ))
            ins.sync_info = mybir.SyncInfo(
                on_wait=[waits[-1]], on_update=list(si.on_update or []))
            changed = True
        new.append(ins)
    if changed:
        b.instructions[:] = new
    for sub in getattr(b, "blocks", []) or []:
        _split_block(sub)


def split_multi_waits(nc):
    for b in nc.main_func.blocks:
        _split_block(b)
    return nc


# ---------------------------------------------------------------------------
import os as _os
_DEBUG_STOP = int(_os.environ.get("KERNEL_DEBUG_STOP", "99"))
_DEBUG_TAIL = int(_os.environ.get("KERNEL_DEBUG_TAIL", "99"))


class _StopBuild(Exception):
    pass


def build():
    nc = bass.Bass()

    X = nc.dram_tensor("x", [C, N], F32, kind="ExternalInput")
    WQ8 = nc.dram_tensor("wq8", [128, CT, C], F8, kind="ExternalInput")
    WK8 = nc.dram_tensor("wk8", [128, CT, C], F8, kind="ExternalInput")
    WV8 = nc.dram_tensor("wv8", [128, CT, C], F8, kind="ExternalInput")
    WPT16 = nc.dram_tensor("wpt16", [C, C], BF16, kind="ExternalInput")
    BQ = nc.dram_tensor("bq2", [128, CT], F32, kind="ExternalInput")
    BK = nc.dram_tensor("bk2", [128, CT], F32, kind="ExternalInput")
    PB2 = nc.dram_tensor("pb2", [128, CT], F32, kind="ExternalInput")
    GAMMA = nc.dram_tensor("gamma2", [128, CT], F32, kind="ExternalInput")
    BETA = nc.dram_tensor("beta2", [128, CT], F32, kind="ExternalInput")
    GMASK = nc.dram_tensor("gmask", [128, 4], F32R, kind="ExternalInput")
    GBCAST = nc.dram_tensor("gbcast", [4, 128], F32R, kind="ExternalInput")
    ONES8 = nc.dram_tensor("ones8", [128, CT, 16], F8, kind="ExternalInput")
    ONESROW = nc.dram_tensor("ones1x128", [1, 128], F32R, kind="ExternalInput")
    EPSC = nc.dram_tensor("epsc", [128, 1], F32, kind="ExternalInput")
    EXPB = nc.dram_tensor("expb", [128, 1], F32, kind="ExternalInput")
    OUT = nc.dram_tensor("out", [C, QH], F32, kind="ExternalOutput")
    dbg = {}
    if _os.environ.get("KERNEL_DEBUG_DUMP"):
        dbg["h8"] = nc.dram_tensor("dbg_h8", [128, CT, N], F8,
                                   kind="ExternalOutput")
        dbg["k8"] = nc.dram_tensor("dbg_k8", [128, CT, N], F8,
                                   kind="ExternalOutput")
        dbg["q8"] = nc.dram_tensor("dbg_q8", [128, CT, QH], F8,
                                   kind="ExternalOutput")
        dbg["vt8"] = nc.dram_tensor("dbg_vt8", [128, MB, C], F8,
                                    kind="ExternalOutput")
        dbg["att"] = nc.dram_tensor("dbg_att", [128, 2, 512], F32,
                                    kind="ExternalOutput")
        dbg["drec"] = nc.dram_tensor("dbg_drec", [1, 512], F32R,
                                     kind="ExternalOutput")
        dbg["exps"] = nc.dram_tensor("dbg_exps", [128, 2, 512], F8,
                                     kind="ExternalOutput")

    handles = dict(X=X, WQ8=WQ8, WK8=WK8, WV8=WV8, WPT16=WPT16, BQ=BQ,
                   BK=BK, PB2=PB2, GAMMA=GAMMA, BETA=BETA, GMASK=GMASK,
                   GBCAST=GBCAST, ONES8=ONES8, ONESROW=ONESROW, EPSC=EPSC,
                   EXPB=EXPB, OUT=OUT)
    try:
        _build_body(nc, dbg, handles)
    except _StopBuild:
        pass
    split_multi_waits(nc)
    return nc


def _build_body(nc, dbg, handles):
    (X, WQ8, WK8, WV8, WPT16, BQ, BK, PB2, GAMMA, BETA, GMASK, GBCAST,
     ONES8, ONESROW, EPSC, EXPB, OUT) = (
        handles[n] for n in (
            "X", "WQ8", "WK8", "WV8", "WPT16", "BQ", "BK", "PB2",
            "GAMMA", "BETA", "GMASK", "GBCAST", "ONES8", "ONESROW",
            "EPSC", "EXPB", "OUT"))
    with tile.TileContext(nc) as tc, nc.allow_low_precision(
            reason="fp8/bf16 attention with fp32 accumulation"):
        with tc.tile_pool(name="big", bufs=1) as big, \
             tc.tile_pool(name="small", bufs=1) as small, \
             tc.tile_pool(name="expp", bufs=3) as expp, \
             tc.tile_pool(name="attp", bufs=2) as attp, \
             tc.tile_pool(name="dbp", bufs=2) as dbp, \
             tc.tile_pool(name="outp", bufs=4) as outp, \
             tc.tile_pool(name="ps_ring", bufs=1, space="PSUM") as ps_ring, \
             tc.tile_pool(name="ps_att", bufs=1, space="PSUM") as ps_attp, \
             tc.tile_pool(name="ps_p", bufs=1, space="PSUM") as ps_pp, \
             tc.tile_pool(name="ps_d", bufs=1, space="PSUM") as ps_dp:

            # ---------------------------------------------- PSUM ring
            ring = ps_ring.tile([128, 4, 512], F32, tag="ring")
            _rctr = [0]

            def rslot():
                i = _rctr[0] % 4
                _rctr[0] += 1
                return ring[:, i, :]

            def rpair():
                if _rctr[0] % 2:
                    _rctr[0] += 1
                i = _rctr[0] % 4
                _rctr[0] += 2
                return ring[:, i:i + 2, :]

            # ------------------------------------------------ load x + weights
            x_sb = [big.tile([128, N], F32, tag=f"x{t}", name=f"x{t}")
                    for t in range(CT)]
            for t in range(CT):
                nc.sync.dma_start(x_sb[t][:], X[t * 128:(t + 1) * 128, :])

            wq8 = small.tile([128, CT, C], F8, tag="wq8")
            wk8 = small.tile([128, CT, C], F8, tag="wk8")
            wv8 = small.tile([128, CT, C], F8, tag="wv8")
            for t_, d_ in ((wq8, WQ8), (wk8, WK8), (wv8, WV8)):
                nc.sync.dma_start(t_[:], d_[:])
            wp16 = [small.tile([128, C], BF16, tag=f"wp{t}", name=f"wp{t}")
                    for t in range(CT)]
            for t in range(CT):
                nc.sync.dma_start(wp16[t][:], WPT16[t * 128:(t + 1) * 128, :])

            bq = small.tile([128, CT], F32, tag="bq")
            bk = small.tile([128, CT], F32, tag="bk")
            pb2 = small.tile([128, CT], F32, tag="pb2")
            gamma = small.tile([128, CT], F32, tag="gamma")
            beta = small.tile([128, CT], F32, tag="beta")
            for t_, d_ in ((bq, BQ), (bk, BK), (pb2, PB2), (gamma, GAMMA),
                           (beta, BETA)):
                nc.sync.dma_start(t_[:], d_[:])
            gmask = small.tile([128, 4], F32R, tag="gmask")
            nc.sync.dma_start(gmask[:], GMASK[:])
            gbcast = small.tile([4, 128], F32R, tag="gbcast")
            nc.sync.dma_start(gbcast[:], GBCAST[:])
            ones8 = small.tile([128, CT, 16], F8, tag="ones8")
            nc.sync.dma_start(ones8[:], ONES8[:])
            onesrow = small.tile([1, 128], F32R, tag="onesrow")
            nc.sync.dma_start(onesrow[:], ONESROW[:])
            epsc = small.tile([128, 1], F32, tag="epsc")
            nc.sync.dma_start(epsc[:], EPSC[:])
            expb = small.tile([128, 1], F32, tag="expb")
            nc.sync.dma_start(expb[:], EXPB[:])

            # ------------------------------------------------ GroupNorm stats
            h8 = big.tile([128, CT, N], F8, tag="h8")
            k8 = big.tile([128, CT, N], F8, tag="k8")
            q8 = big.tile([128, CT, QH], F8, tag="q8")
            vt8 = big.tile([128, MB, C], F8, tag="vt8")

            stats = [small.tile([128, 2], F32R, tag=f"st{t}", name=f"st{t}")
                     for t in range(CT)]
            for t in range(CT):
                nc.vector.tensor_reduce(
                    stats[t][:, 0:1], x_sb[t][:], axis=mybir.AxisListType.X,
                    op=ALU.add)
                # Square output is pure scratch (k8 is overwritten later);
                # the f32 accumulated row-sum is what we keep.
                nc.scalar.activation(
                    k8[:, t, :], x_sb[t][:], AF.Square,
                    accum_out=stats[t][:, 1:2])

            a_c = []
            d_c = []
            for t in range(CT):
                ps_g = rslot()
                nc.tensor.matmul(ps_g[0:4, 0:2], gmask[:], stats[t][:],
                                 start=True, stop=True)
                gstats = small.tile([4, 2], F32R, tag=f"gst{t}", name=f"gst{t}")
                nc.vector.tensor_copy(gstats[:], ps_g[0:4, 0:2])
                ps_bc = rslot()
                nc.tensor.matmul(ps_bc[:, 0:2], gbcast[:], gstats[:],
                                 start=True, stop=True)
                mean = small.tile([128, 1], F32, tag=f"mean{t}")
                ex2 = small.tile([128, 1], F32, tag=f"ex2{t}")
                nc.scalar.mul(mean[:], ps_bc[:, 0:1], INV_CNT)
                nc.scalar.mul(ex2[:], ps_bc[:, 1:2], INV_CNT)
                var = small.tile([128, 1], F32, tag=f"var{t}")
                # var' = mean^2 - ex2; rstd = exp(-0.5*ln(eps - var'))
                nc.vector.scalar_tensor_tensor(
                    var[:], mean[:], mean[:], ex2[:],
                    op0=ALU.mult, op1=ALU.subtract)
                lnv = small.tile([128, 1], F32, tag=f"lnv{t}")
                nc.scalar.activation(lnv[:], var[:], AF.Ln,
                                     bias=epsc[:], scale=-1.0)
                rstd = small.tile([128, 1], F32, tag=f"rstd{t}")
                nc.scalar.activation(rstd[:], lnv[:], AF.Exp,
                                     bias=0.0, scale=-0.5)
                a_t = small.tile([128, 1], F32, tag=f"a{t}")
                nc.vector.tensor_mul(a_t[:], rstd[:], gamma[:, t:t + 1])
                d_t = small.tile([128, 1], F32, tag=f"d{t}")
                tmp = small.tile([128, 1], F32, tag=f"tmp{t}")
                nc.vector.tensor_mul(tmp[:], mean[:], a_t[:])
                nc.vector.tensor_sub(d_t[:], beta[:, t:t + 1], tmp[:])
                a_c.append(a_t)
                d_c.append(d_t)

            # h8 apply: ct0 on ScalarE, ct1 on DVE (parallel engines; the
            # DVE path measures faster for the fp8 store)
            nc.scalar.activation(h8[:, 0, :], x_sb[0][:], AF.Identity,
                                 bias=d_c[0][:], scale=a_c[0][:])
            nc.vector.tensor_scalar(h8[:, 1, :], x_sb[1][:],
                                    a_c[1][:], d_c[1][:],
                                    op0=ALU.mult, op1=ALU.add)

            # ------------------------------------------------ K / Q (fp8 DR)
            if _DEBUG_STOP < 1:
                raise _StopBuild
            # K[ot*128+m, n] = sum_c wk[ot*128+m, c] h[c, n]; PSUM->SBUF
            # bias-add evictions all run on the DVE: one engine draining
            # back-to-back paces the ring better than alternating engines
            # (each handoff costs two sem round-trips and cools the PE).
            def evict_add(dst, ps, bias_col):
                nc.vector.tensor_scalar_add(dst, ps, bias_col)

            for ot in range(CT):
                for cp in range(N // 1024):
                    ps = rpair()
                    for j in range(2):
                        ch = 2 * cp + j
                        nc.tensor.matmul(
                            ps[:, j, :],
                            wk8[:, :, ot * 128:(ot + 1) * 128],
                            h8[:, :, ch * 512:(ch + 1) * 512],
                            start=True, stop=True, perf_mode=DR)
                    evict_add(k8[:, ot, cp * 1024:(cp + 1) * 1024], ps,
                              bk[:, ot:ot + 1])
            for ot in range(CT):
                for cp in range(QH // 1024):
                    ps = rpair()
                    for j in range(2):
                        ch = 2 * cp + j
                        nc.tensor.matmul(
                            ps[:, j, :],
                            wq8[:, :, ot * 128:(ot + 1) * 128],
                            h8[:, :, ch * 512:(ch + 1) * 512],
                            start=True, stop=True, perf_mode=DR)
                    evict_add(q8[:, ot, cp * 1024:(cp + 1) * 1024], ps,
                              bq[:, ot:ot + 1])

            if _DEBUG_STOP < 2:
                raise _StopBuild
            # ------------------------------------------------ V^T (fp8 DR)
            # VT[u*128+m, vc] = sum_c h[c, u*128+m] wv[vc, c]   (bv -> pb2)
            for up in range(NUP):
                ps = rslot()
                for j in range(2):
                    u = 2 * up + j
                    nc.tensor.matmul(
                        ps[:, j * 256:(j + 1) * 256],
                        h8[:, :, u * 128:(u + 1) * 128],
                        wv8[:],
                        start=True, stop=True, perf_mode=DR)
                # V evictions run as a phase-pure Scalar block (DVE still
                # owns the K/Q bias-adds): each engine drains back-to-back.
                nc.scalar.copy(vt8[:, 2 * up:2 * up + 2, :], ps)

            # ------------------------------------------------ attention
            if _DEBUG_STOP < 3:
                raise _StopBuild
            resid = x_sb  # x_sb[ot][:, 0:QH] is this core's residual slice

            # tail state carried from chunk ch-1 into chunk ch's up-loop
            tail = [None]

            def emit_tail(step, ch_prev):
                """step 0: bcast 1/D; 1: proj+final ot0; 2: proj+final ot1.

                The shared ps_p bank serializes bcast -> proj0 -> proj1, but
                each writer only waits on the previous tenant's single DVE
                read, and S^T work is interleaved between steps, so the PE
                stream stays fed."""
                st = tail[0]
                if st is None or st["ch"] != ch_prev:
                    return
                ch = st["ch"]
                qs = slice(ch * 512, (ch + 1) * 512)
                if step == 0:
                    ps_db = ps_pp.tile([128, 512], F32, tag="pp")
                    nc.tensor.matmul(ps_db[:], onesrow[:], st["drec"][:],
                                     start=True, stop=True)
                    db_sb = dbp.tile([128, 512], F32, tag="db")
                    nc.vector.tensor_copy(db_sb[:], ps_db[:])
                    st["db"] = db_sb
                else:
                    ot = step - 1
                    ps_p = ps_pp.tile([128, 512], F32, tag="pp")
                    for ctk in range(CT):
                        nc.tensor.matmul(
                            ps_p[:],
                            wp16[ctk][:, ot * 128:(ot + 1) * 128],
                            st["att"][:, ctk, :],
                            start=(ctk == 0), stop=(ctk == CT - 1))
                    t1 = outp.tile([128, 512], F32, tag="t1")
                    nc.vector.tensor_mul(t1[:], ps_p[:], st["db"][:])
                    o_t = outp.tile([128, 512], F32, tag="o")
                    nc.vector.scalar_tensor_tensor(
                        o_t[:], t1[:], pb2[:, ot:ot + 1],
                        resid[ot][:, qs],
                        op0=ALU.add, op1=ALU.add)
                    nc.sync.dma_start(OUT[ot * 128:(ot + 1) * 128, qs],
                                      o_t[:])
                    if step == 2:
                        tail[0] = None

            # Software-pipelined PE stream: S^T for iteration g+1 is emitted
            # BEFORE PV/D of iteration g, so the PE computes the next scores
            # block while ScalarE runs exp(g) — the exp handoff latency never
            # stalls the (in-order) PE queue.
            def emit_st(g):
                ch, up = divmod(g, NUP)
                qs = slice(ch * 512, (ch + 1) * 512)
                ps_st = rpair()
                for j in range(2):
                    u = 2 * up + j
                    nc.tensor.matmul(
                        ps_st[:, j, :],
                        k8[:, :, u * 128:(u + 1) * 128],
                        q8[:, :, qs],
                        start=True, stop=True, perf_mode=DR)
                return ps_st

            ps_att = None
            ps_d = None
            st_next = emit_st(0) if NCH else None
            for g in range(NCH * NUP):
                ch, up = divmod(g, NUP)
                if up == 0:
                    ps_att = ps_attp.tile([128, 2, 512], F32, tag="psatt")
                    ps_d = ps_dp.tile([1, 512], F32, tag="psd")
                ps_st = st_next
                exps = expp.tile([128, 2, 512], F8, tag="exps")
                nc.scalar.activation(exps[:], ps_st, AF.Exp,
                                     bias=expb[:], scale=SM_SCALE)
                if dbg and g == 0:
                    nc.sync.dma_start(dbg["exps"][:], exps[:])
                if g + 1 < NCH * NUP:
                    st_next = emit_st(g + 1)
                for ct in range(CT):
                    nc.tensor.matmul(
                        ps_att[:, ct, :],
                        vt8[:, 2 * up:2 * up + 2,
                            ct * 128:(ct + 1) * 128],
                        exps[:],
                        start=(up == 0), stop=(up == NUP - 1),
                        perf_mode=DR)
                nc.tensor.matmul(
                    ps_d[:], ones8[:, :, 0:1], exps[:],
                    start=(up == 0), stop=(up == NUP - 1),
                    perf_mode=DR)
                if _DEBUG_STOP >= 5 and up in (3, 5, 7):
                    emit_tail((up - 3) // 2, ch - 1)

                if up != NUP - 1 or _DEBUG_TAIL < 1:
                    continue
                # end of chunk: immediately evict this chunk's accumulators
                att16 = attp.tile([128, 2, 512], BF16, tag="att")
                nc.vector.tensor_copy(att16[:], ps_att[:])
                drec = dbp.tile([1, 512], F32R, tag="drec")
                nc.vector.reciprocal(drec[:], ps_d[:])
                tail[0] = {"ch": ch, "att": att16, "drec": drec}
                if dbg and ch == 0:
                    att32 = attp.tile([128, 2, 512], F32, tag="att32")
                    nc.vector.tensor_copy(att32[:], ps_att[:])
                    nc.sync.dma_start(dbg["att"][:], att32[:])
                    nc.sync.dma_start(dbg["drec"][:], drec[:])
                if _DEBUG_STOP == 4 or (_DEBUG_STOP == 3 and ch == 0):
                    for step in range(min(3, _DEBUG_TAIL - 1)):
                        emit_tail(step, ch)
                if _DEBUG_STOP == 3 and ch == 0:
                    break

            if _DEBUG_STOP >= 5:
                for step in range(3):
                    emit_tail(step, NCH - 1)

            if dbg:
                for name, src in (("h8", h8), ("k8", k8), ("q8", q8),
                                  ("vt8", vt8)):
                    nc.sync.dma_start(dbg[name][:], src[:])


_NC_CACHE = None


def _get_nc():
    global _NC_CACHE
    if _NC_CACHE is None:
        _NC_CACHE = build()
    return _NC_CACHE


def make_in_maps(x, gamma, beta, w_qkv, b_qkv, w_proj, b_proj):
    np_f8 = mybir.dt.np(F8)
    np_bf16 = mybir.dt.np(BF16)
    x = np.asarray(x, np.float32).reshape(B, C, N)
    gamma = np.asarray(gamma, np.float32)
    beta = np.asarray(beta, np.float32)
    w_qkv = np.asarray(w_qkv, np.float32)
    b_qkv = np.asarray(b_qkv, np.float32)
    w_proj = np.asarray(w_proj, np.float32)
    b_proj = np.asarray(b_proj, np.float32)

    wq, wk, wv = w_qkv[0:C], w_qkv[C:2 * C], w_qkv[2 * C:3 * C]
    bq, bk, bv = b_qkv[0:C], b_qkv[C:2 * C], b_qkv[2 * C:3 * C]
    pb2 = (w_proj @ bv + b_proj).astype(np.float32)

    def col2(v):
        return np.ascontiguousarray(v.reshape(CT, 128).T, dtype=np.float32)

    def dr_w(w):
        # [128, CT, C]: out[p, t, o] = w[o, t*128 + p]
        return np.ascontiguousarray(
            w.T.reshape(CT, 128, C).transpose(1, 0, 2)).astype(np_f8)

    gmask = np.zeros((128, 4), np.float32)
    for p in range(128):
        gmask[p, p // 32] = 1.0
    gbcast = np.ascontiguousarray(gmask.T)

    common = {
        "wq8": dr_w(wq),
        "wk8": dr_w(wk),
        "wv8": dr_w(wv),
        "wpt16": np.ascontiguousarray(w_proj.T).astype(np_bf16),
        "bq2": col2(bq),
        "bk2": col2(bk),
        "pb2": col2(pb2),
        "gamma2": col2(gamma),
        "beta2": col2(beta),
        "gmask": gmask,
        "gbcast": gbcast,
        "ones8": np.ones((128, CT, 16), np_f8),
        "ones1x128": np.ones((1, 128), np.float32),
        "epsc": np.full((128, 1), EPS, np.float32),
        "expb": np.full((128, 1), EXP_BIAS, np.float32),
    }

    in_maps = []
    for core in range(NCORES):
        b, half = core // 2, core % 2
        qoff = half * QH
        xc = np.concatenate([x[b][:, qoff:], x[b][:, :qoff]], axis=1)
        m = dict(common)
        m["x"] = np.ascontiguousarray(xc)
        in_maps.append(m)
    return in_maps


def gather_out(results):
    out = np.empty((B, C, N), np.float32)
    for core in range(NCORES):
        b, half = core // 2, core % 2
        qoff = half * QH
        out[b][:, qoff:qoff + QH] = results[core]["out"]
    return out.reshape(B, C, HH, WW)


def kernel(x, gamma, beta, w_qkv, b_qkv, w_proj, b_proj, **run_kwargs):
    nc = _get_nc()
    in_maps = make_in_maps(x, gamma, beta, w_qkv, b_qkv, w_proj, b_proj)
    res = run_bass_kernel_spmd(nc, in_maps, core_ids=list(range(NCORES)),
                               **run_kwargs)
    out = gather_out(res.results)
    kernel.last_results = res
    return out


# revision 51
# speedup vs baseline: 1.1060x; 1.0624x over previous
"""AttentionBlock (GroupNorm + QKV 1x1conv + full attention + proj + residual)
for Trainium2, data-parallel over (batch, query-half) across 8 NeuronCores.

Self-contained: hardcodes shapes B=4, C=256, H=W=64 from the problem spec.

Per-core plan (core c: batch b=c//2, query-half h=c%2):
  - Host rolls x[b]'s pixel axis so this core's 2048 queries are columns
    0:2048 (attention is permutation-invariant over keys, and GroupNorm
    stats are permutation-invariant, so a single SPMD program serves all
    cores).
  - GroupNorm -> h8 (fp8e4, per-channel affine folded into one activation).
  - All large matmuls run in fp8e4 with perf_mode=DoubleRow (2 fp8 MACs
    per PE per cycle, K=256 contracted per pass):
      K/Q = w.T @ h (biases added on the DVE during PSUM->SBUF eviction),
      VT[m, c] built directly transposed, S^T = K.T Q, PV, and the softmax
      denominator D = ones.T @ exp(S^T/16 - 2).
  - softmax: unnormalized exp on ScalarE (fp8 output); 1/D via
    nc.vector.reciprocal (DVE), broadcast across partitions with a
    1-row f32r matmul; normalization folded AFTER the bf16 projection
    (proj(att)*diag(1/D) == proj(att*diag(1/D))), which takes the slow
    reciprocal off the PE critical path.
  - out = x + proj(PV)/D + pb2, with pb2 = w_proj@bv + b_proj.

PSUM: one 4-slot ring [128,4,512] (paired allocations only) for S^T and
K/Q/V production, a persistent [128,2,512] PV accumulator, a [1,512]
denominator accumulator, and one [128,512] bank shared by the broadcast
and projection. The PE stream is software-pipelined: S^T for iteration
g+1 is emitted before PV/D of iteration g (across chunk boundaries), so
the exp handoff latency never stalls the in-order PE queue; per-chunk
tails (1/D broadcast, proj+final per ot) are emitted 3-7 iterations into
the NEXT chunk's stream, after the DVE reciprocal has drained.

The toolchain's walrus build accepts only one sync-wait per instruction,
so a post-pass splits multi-wait instructions into NoOp chains.
"""

import sys

if "/opt/trn_rl_repo" not in sys.path:
    sys.path.insert(0, "/opt/trn_rl_repo")

import numpy as np

import concourse.bass as bass
import concourse.mybir as mybir
import concourse.tile as tile
from concourse.bass_utils import run_bass_kernel_spmd

F32 = mybir.dt.float32
F32R = mybir.dt.float32r
F8 = mybir.dt.float8e4
BF16 = mybir.dt.bfloat16
AF = mybir.ActivationFunctionType
ALU = mybir.AluOpType
DR = mybir.MatmulPerfMode.DoubleRow

B, C, HH, WW = 4, 256, 64, 64
N = HH * WW          # 4096 pixels
G = 8                # groups
QH = N // 2          # queries per core
NCORES = 8
EPS = 1e-5
INV_CNT = 1.0 / (32 * N)   # 1 / elements per group
SM_SCALE = 1.0 / 16.0      # 1/sqrt(C)
EXP_BIAS = -3.0            # exp(s - 3): keeps fp8 exps inside e4m3 range
                           # (max observed logit ~8.0 + fp8 q/k noise; the
                           # shift cancels exactly in the softmax ratio).
                           # More negative wastes e4m3 range on subnormals,
                           # which measurably slows the fp8 datapath.

CT = C // 128        # 2 channel tiles
MB = N // 128        # 32 key blocks
NCH = QH // 512      # 4 query chunks per core
NUP = MB // 2        # 16 key-block pairs


# ---------------------------------------------------------------------------
# walrus in this env allows only ONE sync-wait command per instruction.
# Split multi-wait instructions into single-wait NoOps (same engine, so
# in-order execution preserves the blocking semantics exactly).
_ws_counter = [0]


def _split_block(b):
    new = []
    changed = False
    for ins in b.instructions:
        si = ins.sync_info
        if si is not None and si.on_wait and len(si.on_wait) > 1:
            waits = list(si.on_wait)
            for w in waits[:-1]:
                _ws_counter[0] += 1
                new.append(mybir.InstNoOp(
                    name=f"I-waitsplit-{_ws_counter[0]}",
                    engine=ins.engine,
                    sync_info=mybir.SyncInfo(on_wait=[w], on_update=[]),
                ))
            ins.sync_info = mybir.SyncInfo(
                on_wait=[waits[-1]], on_update=list(si.on_update or []))
            changed = True
        new.append(ins)
    if changed:
        b.instructions[:] = new
    for sub in getattr(b, "blocks", []) or []:
        _split_block(sub)


def split_multi_waits(nc):
    for b in nc.main_func.blocks:
        _split_block(b)
    return nc


# ---------------------------------------------------------------------------
import os as _os
_DEBUG_STOP = int(_os.environ.get("KERNEL_DEBUG_STOP", "99"))
_DEBUG_TAIL = int(_os.environ.get("KERNEL_DEBUG_TAIL", "99"))


class _StopBuild(Exception):
    pass


def build():
    nc = bass.Bass()

    X = nc.dram_tensor("x", [C, N], F32, kind="ExternalInput")
    WQ8 = nc.dram_tensor("wq8", [128, CT, C], F8, kind="ExternalInput")
    WK8 = nc.dram_tensor("wk8", [128, CT, C], F8, kind="ExternalInput")
    WV8 = nc.dram_tensor("wv8", [128, CT, C], F8, kind="ExternalInput")
    WPT16 = nc.dram_tensor("wpt16", [C, C], BF16, kind="ExternalInput")
    BQ = nc.dram_tensor("bq2", [128, CT], F32, kind="ExternalInput")
    BK = nc.dram_tensor("bk2", [128, CT], F32, kind="ExternalInput")
    PB2 = nc.dram_tensor("pb2", [128, CT], F32, kind="ExternalInput")
    GAMMA = nc.dram_tensor("gamma2", [128, CT], F32, kind="ExternalInput")
    BETA = nc.dram_tensor("beta2", [128, CT], F32, kind="ExternalInput")
    GMASK = nc.dram_tensor("gmask", [128, 4], F32R, kind="ExternalInput")
    GBCAST = nc.dram_tensor("gbcast", [4, 128], F32R, kind="ExternalInput")
    ONES8 = nc.dram_tensor("ones8", [128, CT, 16], F8, kind="ExternalInput")
    ONESROW = nc.dram_tensor("ones1x128", [1, 128], F32R, kind="ExternalInput")
    EPSC = nc.dram_tensor("epsc", [128, 1], F32, kind="ExternalInput")
    EXPB = nc.dram_tensor("expb", [128, 1], F32, kind="ExternalInput")
    OUT = nc.dram_tensor("out", [C, QH], F32, kind="ExternalOutput")
    dbg = {}
    if _os.environ.get("KERNEL_DEBUG_DUMP"):
        dbg["h8"] = nc.dram_tensor("dbg_h8", [128, CT, N], F8,
                                   kind="ExternalOutput")
        dbg["k8"] = nc.dram_tensor("dbg_k8", [128, CT, N], F8,
                                   kind="ExternalOutput")
        dbg["q8"] = nc.dram_tensor("dbg_q8", [128, CT, QH], F8,
                                   kind="ExternalOutput")
        dbg["vt8"] = nc.dram_tensor("dbg_vt8", [128, MB, C], F8,
                                    kind="ExternalOutput")
        dbg["att"] = nc.dram_tensor("dbg_att", [128, 2, 512], F32,
                                    kind="ExternalOutput")
        dbg["drec"] = nc.dram_tensor("dbg_drec", [1, 512], F32R,
                                     kind="ExternalOutput")
        dbg["exps"] = nc.dram_tensor("dbg_exps", [128, 2, 512], F8,
                                     kind="ExternalOutput")

    handles = dict(X=X, WQ8=WQ8, WK8=WK8, WV8=WV8, WPT16=WPT16, BQ=BQ,
                   BK=BK, PB2=PB2, GAMMA=GAMMA, BETA=BETA, GMASK=GMASK,
                   GBCAST=GBCAST, ONES8=ONES8, ONESROW=ONESROW, EPSC=EPSC,
                   EXPB=EXPB, OUT=OUT)
    try:
        _build_body(nc, dbg, handles)
    except _StopBuild:
        pass
    split_multi_waits(nc)
    return nc


def _build_body(nc, dbg, handles):
    (X, WQ8, WK8, WV8, WPT16, BQ, BK, PB2, GAMMA, BETA, GMASK, GBCAST,
     ONES8, ONESROW, EPSC, EXPB, OUT) = (
        handles[n] for n in (
            "X", "WQ8", "WK8", "WV8", "WPT16", "BQ", "BK", "PB2",
            "GAMMA", "BETA", "GMASK", "GBCAST", "ONES8", "ONESROW",
            "EPSC", "EXPB", "OUT"))
    with tile.TileContext(nc) as tc, nc.allow_low_precision(
            reason="fp8/bf16 attention with fp32 accumulation"):
        with tc.tile_pool(name="big", bufs=1) as big, \
             tc.tile_pool(name="small", bufs=1) as small, \
             tc.tile_pool(name="expp", bufs=3) as expp, \
             tc.tile_pool(name="attp", bufs=2) as attp, \
             tc.tile_pool(name="dbp", bufs=2) as dbp, \
             tc.tile_pool(name="outp", bufs=4) as outp, \
             tc.tile_pool(name="ps_ring", bufs=1, space="PSUM") as ps_ring, \
             tc.tile_pool(name="ps_att", bufs=1, space="PSUM") as ps_attp, \
             tc.tile_pool(name="ps_p", bufs=1, space="PSUM") as ps_pp, \
             tc.tile_pool(name="ps_d", bufs=1, space="PSUM") as ps_dp:

            # ---------------------------------------------- PSUM ring
            ring = ps_ring.tile([128, 4, 512], F32, tag="ring")
            _rctr = [0]

            def rslot():
                i = _rctr[0] % 4
                _rctr[0] += 1
                return ring[:, i, :]

            def rpair():
                if _rctr[0] % 2:
                    _rctr[0] += 1
                i = _rctr[0] % 4
                _rctr[0] += 2
                return ring[:, i:i + 2, :]

            # ------------------------------------------------ load x + weights
            x_sb = [big.tile([128, N], F32, tag=f"x{t}", name=f"x{t}")
                    for t in range(CT)]
            for t in range(CT):
                nc.sync.dma_start(x_sb[t][:], X[t * 128:(t + 1) * 128, :])

            wq8 = small.tile([128, CT, C], F8, tag="wq8")
            wk8 = small.tile([128, CT, C], F8, tag="wk8")
            wv8 = small.tile([128, CT, C], F8, tag="wv8")
            for t_, d_ in ((wq8, WQ8), (wk8, WK8), (wv8, WV8)):
                nc.sync.dma_start(t_[:], d_[:])
            wp16 = [small.tile([128, C], BF16, tag=f"wp{t}", name=f"wp{t}")
                    for t in range(CT)]
            for t in range(CT):
                nc.sync.dma_start(wp16[t][:], WPT16[t * 128:(t + 1) * 128, :])

            bq = small.tile([128, CT], F32, tag="bq")
            bk = small.tile([128, CT], F32, tag="bk")
            pb2 = small.tile([128, CT], F32, tag="pb2")
            gamma = small.tile([128, CT], F32, tag="gamma")
            beta = small.tile([128, CT], F32, tag="beta")
            for t_, d_ in ((bq, BQ), (bk, BK), (pb2, PB2), (gamma, GAMMA),
                           (beta, BETA)):
                nc.sync.dma_start(t_[:], d_[:])
            gmask = small.tile([128, 4], F32R, tag="gmask")
            nc.sync.dma_start(gmask[:], GMASK[:])
            gbcast = small.tile([4, 128], F32R, tag="gbcast")
            nc.sync.dma_start(gbcast[:], GBCAST[:])
            ones8 = small.tile([128, CT, 16], F8, tag="ones8")
            nc.sync.dma_start(ones8[:], ONES8[:])
            onesrow = small.tile([1, 128], F32R, tag="onesrow")
            nc.sync.dma_start(onesrow[:], ONESROW[:])
            epsc = small.tile([128, 1], F32, tag="epsc")
            nc.sync.dma_start(epsc[:], EPSC[:])
            expb = small.tile([128, 1], F32, tag="expb")
            nc.sync.dma_start(expb[:], EXPB[:])

            # ------------------------------------------------ GroupNorm stats
            h8 = big.tile([128, CT, N], F8, tag="h8")
            k8 = big.tile([128, CT, N], F8, tag="k8")
            q8 = big.tile([128, CT, QH], F8, tag="q8")
            vt8 = big.tile([128, MB, C], F8, tag="vt8")

            stats = [small.tile([128, 2], F32R, tag=f"st{t}", name=f"st{t}")
                     for t in range(CT)]
            for t in range(CT):
                nc.vector.tensor_reduce(
                    stats[t][:, 0:1], x_sb[t][:], axis=mybir.AxisListType.X,
                    op=ALU.add)
                # Square output is pure scratch (k8 is overwritten later);
                # the f32 accumulated row-sum is what we keep.
                nc.scalar.activation(
                    k8[:, t, :], x_sb[t][:], AF.Square,
                    accum_out=stats[t][:, 1:2])

            a_c = []
            d_c = []
            for t in range(CT):
                ps_g = rslot()
                nc.tensor.matmul(ps_g[0:4, 0:2], gmask[:], stats[t][:],
                                 start=True, stop=True)
                gstats = small.tile([4, 2], F32R, tag=f"gst{t}", name=f"gst{t}")
                nc.vector.tensor_copy(gstats[:], ps_g[0:4, 0:2])
                ps_bc = rslot()
                nc.tensor.matmul(ps_bc[:, 0:2], gbcast[:], gstats[:],
                                 start=True, stop=True)
                mean = small.tile([128, 1], F32, tag=f"mean{t}")
                ex2 = small.tile([128, 1], F32, tag=f"ex2{t}")
                nc.scalar.mul(mean[:], ps_bc[:, 0:1], INV_CNT)
                nc.scalar.mul(ex2[:], ps_bc[:, 1:2], INV_CNT)
                var = small.tile([128, 1], F32, tag=f"var{t}")
                # var' = mean^2 - ex2; rstd = exp(-0.5*ln(eps - var'))
                nc.vector.scalar_tensor_tensor(
                    var[:], mean[:], mean[:], ex2[:],
                    op0=ALU.mult, op1=ALU.subtract)
                lnv = small.tile([128, 1], F32, tag=f"lnv{t}")
                nc.scalar.activation(lnv[:], var[:], AF.Ln,
                                     bias=epsc[:], scale=-1.0)
                rstd = small.tile([128, 1], F32, tag=f"rstd{t}")
                nc.scalar.activation(rstd[:], lnv[:], AF.Exp,
                                     bias=0.0, scale=-0.5)
                a_t = small.tile([128, 1], F32, tag=f"a{t}")
                nc.vector.tensor_mul(a_t[:], rstd[:], gamma[:, t:t + 1])
                d_t = small.tile([128, 1], F32, tag=f"d{t}")
                tmp = small.tile([128, 1], F32, tag=f"tmp{t}")
                nc.vector.tensor_mul(tmp[:], mean[:], a_t[:])
                nc.vector.tensor_sub(d_t[:], beta[:, t:t + 1], tmp[:])
                a_c.append(a_t)
                d_c.append(d_t)

            # h8 apply: ct0 on ScalarE, ct1 on DVE (parallel engines; the
            # DVE path measures faster for the fp8 store)
            nc.scalar.activation(h8[:, 0, :], x_sb[0][:], AF.Identity,
                                 bias=d_c[0][:], scale=a_c[0][:])
            nc.vector.tensor_scalar(h8[:, 1, :], x_sb[1][:],
                                    a_c[1][:], d_c[1][:],
                                    op0=ALU.mult, op1=ALU.add)

            # ------------------------------------------------ K / Q (fp8 DR)
            if _DEBUG_STOP < 1:
                raise _StopBuild
            # K[ot*128+m, n] = sum_c wk[ot*128+m, c] h[c, n]; PSUM->SBUF
            # bias-add evictions all run on the DVE: one engine draining
            # back-to-back paces the ring better than alternating engines
            # (each handoff costs two sem round-trips and cools the PE).
            def evict_add(dst, ps, bias_col):
                nc.vector.tensor_scalar_add(dst, ps, bias_col)

            # During production the attention accumulators (ps_att, ps_p)
            # are idle — borrow them as extra pipeline slots so the
            # evictions stream back-to-back instead of lockstepping
            # MM -> evict -> MM on a 2-pair ring.
            _pctr = [0]

            def prod_pair():
                i = _pctr[0] % 3
                _pctr[0] += 1
                if i == 2:
                    pa = ps_attp.tile([128, 2, 512], F32, tag="psatt",
                                      name="prodpa")
                    return pa[:, :, :]
                return rpair()

            _vctr = [0]

            def prod_slot():
                i = _vctr[0] % 5
                _vctr[0] += 1
                if i == 4:
                    pv = ps_pp.tile([128, 512], F32, tag="pp",
                                    name="prodpv")
                    return pv[:, :]
                return rslot()

            for ot in range(CT):
                for cp in range(N // 1024):
                    ps = prod_pair()
                    for j in range(2):
                        ch = 2 * cp + j
                        nc.tensor.matmul(
                            ps[:, j, :],
                            wk8[:, :, ot * 128:(ot + 1) * 128],
                            h8[:, :, ch * 512:(ch + 1) * 512],
                            start=True, stop=True, perf_mode=DR)
                    evict_add(k8[:, ot, cp * 1024:(cp + 1) * 1024], ps,
                              bk[:, ot:ot + 1])
            for ot in range(CT):
                for cp in range(QH // 1024):
                    ps = prod_pair()
                    for j in range(2):
                        ch = 2 * cp + j
                        nc.tensor.matmul(
                            ps[:, j, :],
                            wq8[:, :, ot * 128:(ot + 1) * 128],
                            h8[:, :, ch * 512:(ch + 1) * 512],
                            start=True, stop=True, perf_mode=DR)
                    evict_add(q8[:, ot, cp * 1024:(cp + 1) * 1024], ps,
                              bq[:, ot:ot + 1])

            if _DEBUG_STOP < 2:
                raise _StopBuild
            # ------------------------------------------------ V^T (fp8 DR)
            # VT[u*128+m, vc] = sum_c h[c, u*128+m] wv[vc, c]   (bv -> pb2)
            for up in range(NUP):
                ps = prod_slot()
                for j in range(2):
                    u = 2 * up + j
                    nc.tensor.matmul(
                        ps[:, j * 256:(j + 1) * 256],
                        h8[:, :, u * 128:(u + 1) * 128],
                        wv8[:],
                        start=True, stop=True, perf_mode=DR)
                # V evictions run as a phase-pure Scalar block (DVE still
                # owns the K/Q bias-adds): each engine drains back-to-back.
                nc.scalar.copy(vt8[:, 2 * up:2 * up + 2, :], ps)

            # ------------------------------------------------ attention
            if _DEBUG_STOP < 3:
                raise _StopBuild
            resid = x_sb  # x_sb[ot][:, 0:QH] is this core's residual slice

            # tail state carried from chunk ch-1 into chunk ch's up-loop
            tail = [None]

            def emit_tail(step, ch_prev):
                """step 0: bcast 1/D; 1: proj+final ot0; 2: proj+final ot1.

                The shared ps_p bank serializes bcast -> proj0 -> proj1, but
                each writer only waits on the previous tenant's single DVE
                read, and S^T work is interleaved between steps, so the PE
                stream stays fed."""
                st = tail[0]
                if st is None or st["ch"] != ch_prev:
                    return
                ch = st["ch"]
                qs = slice(ch * 512, (ch + 1) * 512)
                if step == 0:
                    ps_db = ps_pp.tile([128, 512], F32, tag="pp")
                    nc.tensor.matmul(ps_db[:], onesrow[:], st["drec"][:],
                                     start=True, stop=True)
                    db_sb = dbp.tile([128, 512], F32, tag="db")
                    nc.vector.tensor_copy(db_sb[:], ps_db[:])
                    st["db"] = db_sb
                else:
                    ot = step - 1
                    ps_p = ps_pp.tile([128, 512], F32, tag="pp")
                    for ctk in range(CT):
                        nc.tensor.matmul(
                            ps_p[:],
                            wp16[ctk][:, ot * 128:(ot + 1) * 128],
                            st["att"][:, ctk, :],
                            start=(ctk == 0), stop=(ctk == CT - 1))
                    t1 = outp.tile([128, 512], F32, tag="t1")
                    nc.vector.tensor_mul(t1[:], ps_p[:], st["db"][:])
                    o_t = outp.tile([128, 512], F32, tag="o")
                    nc.vector.scalar_tensor_tensor(
                        o_t[:], t1[:], pb2[:, ot:ot + 1],
                        resid[ot][:, qs],
                        op0=ALU.add, op1=ALU.add)
                    nc.sync.dma_start(OUT[ot * 128:(ot + 1) * 128, qs],
                                      o_t[:])
                    if step == 2:
                        tail[0] = None

            # Software-pipelined PE stream: S^T for iteration g+1 is emitted
            # BEFORE PV/D of iteration g, so the PE computes the next scores
            # block while ScalarE runs exp(g) — the exp handoff latency never
            # stalls the (in-order) PE queue.
            def emit_st(g):
                ch, up = divmod(g, NUP)
                qs = slice(ch * 512, (ch + 1) * 512)
                ps_st = rpair()
                for j in range(2):
                    u = 2 * up + j
                    nc.tensor.matmul(
                        ps_st[:, j, :],
                        k8[:, :, u * 128:(u + 1) * 128],
                        q8[:, :, qs],
                        start=True, stop=True, perf_mode=DR)
                return ps_st

            ps_att = None
            ps_d = None
            st_next = emit_st(0) if NCH else None
            for g in range(NCH * NUP):
                ch, up = divmod(g, NUP)
                if up == 0:
                    ps_att = ps_attp.tile([128, 2, 512], F32, tag="psatt")
                    ps_d = ps_dp.tile([1, 512], F32, tag="psd")
                ps_st = st_next
                exps = expp.tile([128, 2, 512], F8, tag="exps")
                nc.scalar.activation(exps[:], ps_st, AF.Exp,
                                     bias=expb[:], scale=SM_SCALE)
                if dbg and g == 0:
                    nc.sync.dma_start(dbg["exps"][:], exps[:])
                if g + 1 < NCH * NUP:
                    st_next = emit_st(g + 1)
                for ct in range(CT):
                    nc.tensor.matmul(
                        ps_att[:, ct, :],
                        vt8[:, 2 * up:2 * up + 2,
                            ct * 128:(ct + 1) * 128],
                        exps[:],
                        start=(up == 0), stop=(up == NUP - 1),
                        perf_mode=DR)
                nc.tensor.matmul(
                    ps_d[:], ones8[:, :, 0:1], exps[:],
                    start=(up == 0), stop=(up == NUP - 1),
                    perf_mode=DR)
                if _DEBUG_STOP >= 5 and up in (3, 5, 7):
                    emit_tail((up - 3) // 2, ch - 1)

                if up != NUP - 1 or _DEBUG_TAIL < 1:
                    continue
                # end of chunk: immediately evict this chunk's accumulators
                att16 = attp.tile([128, 2, 512], BF16, tag="att")
                nc.vector.tensor_copy(att16[:], ps_att[:])
                drec = dbp.tile([1, 512], F32R, tag="drec")
                nc.vector.reciprocal(drec[:], ps_d[:])
                tail[0] = {"ch": ch, "att": att16, "drec": drec}
                if dbg and ch == 0:
                    att32 = attp.tile([128, 2, 512], F32, tag="att32")
                    nc.vector.tensor_copy(att32[:], ps_att[:])
                    nc.sync.dma_start(dbg["att"][:], att32[:])
                    nc.sync.dma_start(dbg["drec"][:], drec[:])
                if _DEBUG_STOP == 4 or (_DEBUG_STOP == 3 and ch == 0):
                    for step in range(min(3, _DEBUG_TAIL - 1)):
                        emit_tail(step, ch)
                if _DEBUG_STOP == 3 and ch == 0:
                    break

            if _DEBUG_STOP >= 5:
                for step in range(3):
                    emit_tail(step, NCH - 1)

            if dbg:
                for name, src in (("h8", h8), ("k8", k8), ("q8", q8),
                                  ("vt8", vt8)):
                    nc.sync.dma_start(dbg[name][:], src[:])


_NC_CACHE = None


def _get_nc():
    global _NC_CACHE
    if _NC_CACHE is None:
        _NC_CACHE = build()
    return _NC_CACHE


def make_in_maps(x, gamma, beta, w_qkv, b_qkv, w_proj, b_proj):
    np_f8 = mybir.dt.np(F8)
    np_bf16 = mybir.dt.np(BF16)
    x = np.asarray(x, np.float32).reshape(B, C, N)
    gamma = np.asarray(gamma, np.float32)
    beta = np.asarray(beta, np.float32)
    w_qkv = np.asarray(w_qkv, np.float32)
    b_qkv = np.asarray(b_qkv, np.float32)
    w_proj = np.asarray(w_proj, np.float32)
    b_proj = np.asarray(b_proj, np.float32)

    wq, wk, wv = w_qkv[0:C], w_qkv[C:2 * C], w_qkv[2 * C:3 * C]
    bq, bk, bv = b_qkv[0:C], b_qkv[C:2 * C], b_qkv[2 * C:3 * C]
    pb2 = (w_proj @ bv + b_proj).astype(np.float32)

    def col2(v):
        return np.ascontiguousarray(v.reshape(CT, 128).T, dtype=np.float32)

    def dr_w(w):
        # [128, CT, C]: out[p, t, o] = w[o, t*128 + p]
        return np.ascontiguousarray(
            w.T.reshape(CT, 128, C).transpose(1, 0, 2)).astype(np_f8)

    gmask = np.zeros((128, 4), np.float32)
    for p in range(128):
        gmask[p, p // 32] = 1.0
    gbcast = np.ascontiguousarray(gmask.T)

    common = {
        "wq8": dr_w(wq),
        "wk8": dr_w(wk),
        "wv8": dr_w(wv),
        "wpt16": np.ascontiguousarray(w_proj.T).astype(np_bf16),
        "bq2": col2(bq),
        "bk2": col2(bk),
        "pb2": col2(pb2),
        "gamma2": col2(gamma),
        "beta2": col2(beta),
        "gmask": gmask,
        "gbcast": gbcast,
        "ones8": np.ones((128, CT, 16), np_f8),
        "ones1x128": np.ones((1, 128), np.float32),
        "epsc": np.full((128, 1), EPS, np.float32),
        "expb": np.full((128, 1), EXP_BIAS, np.float32),
    }

    in_maps = []
    for core in range(NCORES):
        b, half = core // 2, core % 2
        qoff = half * QH
        xc = np.concatenate([x[b][:, qoff:], x[b][:, :qoff]], axis=1)
        m = dict(common)
        m["x"] = np.ascontiguousarray(xc)
        in_maps.append(m)
    return in_maps


def gather_out(results):
    out = np.empty((B, C, N), np.float32)
    for core in range(NCORES):
        b, half = core // 2, core % 2
        qoff = half * QH
        out[b][:, qoff:qoff + QH] = results[core]["out"]
    return out.reshape(B, C, HH, WW)


def kernel(x, gamma, beta, w_qkv, b_qkv, w_proj, b_proj, **run_kwargs):
    nc = _get_nc()
    in_maps = make_in_maps(x, gamma, beta, w_qkv, b_qkv, w_proj, b_proj)
    res = run_bass_kernel_spmd(nc, in_maps, core_ids=list(range(NCORES)),
                               **run_kwargs)
    out = gather_out(res.results)
    kernel.last_results = res
    return out


# revision 53
# speedup vs baseline: 1.1157x; 1.0088x over previous
"""AttentionBlock (GroupNorm + QKV 1x1conv + full attention + proj + residual)
for Trainium2, data-parallel over (batch, query-half) across 8 NeuronCores.

Self-contained: hardcodes shapes B=4, C=256, H=W=64 from the problem spec.

Per-core plan (core c: batch b=c//2, query-half h=c%2):
  - Host rolls x[b]'s pixel axis so this core's 2048 queries are columns
    0:2048 (attention is permutation-invariant over keys, and GroupNorm
    stats are permutation-invariant, so a single SPMD program serves all
    cores).
  - GroupNorm -> h8 (fp8e4, per-channel affine folded into one activation).
  - All large matmuls run in fp8e4 with perf_mode=DoubleRow (2 fp8 MACs
    per PE per cycle, K=256 contracted per pass):
      K/Q = w.T @ h (biases added on the DVE during PSUM->SBUF eviction),
      VT[m, c] built directly transposed, S^T = K.T Q, PV, and the softmax
      denominator D = ones.T @ exp(S^T/16 - 2).
  - softmax: unnormalized exp on ScalarE (fp8 output); 1/D via
    nc.vector.reciprocal (DVE), broadcast across partitions with a
    1-row f32r matmul; normalization folded AFTER the bf16 projection
    (proj(att)*diag(1/D) == proj(att*diag(1/D))), which takes the slow
    reciprocal off the PE critical path.
  - out = x + proj(PV)/D + pb2, with pb2 = w_proj@bv + b_proj.

PSUM: one 4-slot ring [128,4,512] (paired allocations only) for S^T and
K/Q/V production, a persistent [128,2,512] PV accumulator, a [1,512]
denominator accumulator, and one [128,512] bank shared by the broadcast
and projection. The PE stream is software-pipelined: S^T for iteration
g+1 is emitted before PV/D of iteration g (across chunk boundaries), so
the exp handoff latency never stalls the in-order PE queue; per-chunk
tails (1/D broadcast, proj+final per ot) are emitted 3-7 iterations into
the NEXT chunk's stream, after the DVE reciprocal has drained.

The toolchain's walrus build accepts only one sync-wait per instruction,
so a post-pass splits multi-wait instructions into NoOp chains.
"""

import sys

if "/opt/trn_rl_repo" not in sys.path:
    sys.path.insert(0, "/opt/trn_rl_repo")

import numpy as np

import concourse.bass as bass
import concourse.mybir as mybir
import concourse.tile as tile
from concourse.bass_utils import run_bass_kernel_spmd

F32 = mybir.dt.float32
F32R = mybir.dt.float32r
F8 = mybir.dt.float8e4
BF16 = mybir.dt.bfloat16
AF = mybir.ActivationFunctionType
ALU = mybir.AluOpType
DR = mybir.MatmulPerfMode.DoubleRow

B, C, HH, WW = 4, 256, 64, 64
N = HH * WW          # 4096 pixels
G = 8                # groups
QH = N // 2          # queries per core
NCORES = 8
EPS = 1e-5
INV_CNT = 1.0 / (32 * N)   # 1 / elements per group
SM_SCALE = 1.0 / 16.0      # 1/sqrt(C)
EXP_BIAS = -3.0            # exp(s - 3): keeps fp8 exps inside e4m3 range
                           # (max observed logit ~8.0 + fp8 q/k noise; the
                           # shift cancels exactly in the softmax ratio).
                           # More negative wastes e4m3 range on subnormals,
                           # which measurably slows the fp8 datapath.

CT = C // 128        # 2 channel tiles
MB = N // 128        # 32 key blocks
NCH = QH // 512      # 4 query chunks per core
NUP = MB // 2        # 16 key-block pairs


# ---------------------------------------------------------------------------
# walrus in this env allows only ONE sync-wait command per instruction.
# Split multi-wait instructions into single-wait NoOps (same engine, so
# in-order execution preserves the blocking semantics exactly).
_ws_counter = [0]


def _split_block(b):
    new = []
    changed = False
    for ins in b.instructions:
        si = ins.sync_info
        if si is not None and si.on_wait and len(si.on_wait) > 1:
            waits = list(si.on_wait)
            for w in waits[:-1]:
                _ws_counter[0] += 1
                new.append(mybir.InstNoOp(
                    name=f"I-waitsplit-{_ws_counter[0]}",
                    engine=ins.engine,
                    sync_info=mybir.SyncInfo(on_wait=[w], on_update=[]),
                ))
            ins.sync_info = mybir.SyncInfo(
                on_wait=[waits[-1]], on_update=list(si.on_update or []))
            changed = True
        new.append(ins)
    if changed:
        b.instructions[:] = new
    for sub in getattr(b, "blocks", []) or []:
        _split_block(sub)


def split_multi_waits(nc):
    for b in nc.main_func.blocks:
        _split_block(b)
    return nc


# ---------------------------------------------------------------------------
import os as _os
_DEBUG_STOP = int(_os.environ.get("KERNEL_DEBUG_STOP", "99"))
_DEBUG_TAIL = int(_os.environ.get("KERNEL_DEBUG_TAIL", "99"))


class _StopBuild(Exception):
    pass


def build():
    nc = bass.Bass()

    X = nc.dram_tensor("x", [C, N], F32, kind="ExternalInput")
    WQ8 = nc.dram_tensor("wq8", [128, CT, C], F8, kind="ExternalInput")
    WK8 = nc.dram_tensor("wk8", [128, CT, C], F8, kind="ExternalInput")
    WV8 = nc.dram_tensor("wv8", [128, CT, C], F8, kind="ExternalInput")
    WPT16 = nc.dram_tensor("wpt16", [C, C], BF16, kind="ExternalInput")
    BQ = nc.dram_tensor("bq2", [128, CT], F32, kind="ExternalInput")
    BK = nc.dram_tensor("bk2", [128, CT], F32, kind="ExternalInput")
    PB2 = nc.dram_tensor("pb2", [128, CT], F32, kind="ExternalInput")
    GAMMA = nc.dram_tensor("gamma2", [128, CT], F32, kind="ExternalInput")
    BETA = nc.dram_tensor("beta2", [128, CT], F32, kind="ExternalInput")
    GMASK = nc.dram_tensor("gmask", [128, 4], F32R, kind="ExternalInput")
    GBCAST = nc.dram_tensor("gbcast", [4, 128], F32R, kind="ExternalInput")
    ONES8 = nc.dram_tensor("ones8", [128, CT, 16], F8, kind="ExternalInput")
    ONESROW = nc.dram_tensor("ones1x128", [1, 128], F32R, kind="ExternalInput")
    EPSC = nc.dram_tensor("epsc", [128, 1], F32, kind="ExternalInput")
    EXPB = nc.dram_tensor("expb", [128, 1], F32, kind="ExternalInput")
    OUT = nc.dram_tensor("out", [C, QH], F32, kind="ExternalOutput")
    dbg = {}
    if _os.environ.get("KERNEL_DEBUG_DUMP"):
        dbg["h8"] = nc.dram_tensor("dbg_h8", [128, CT, N], F8,
                                   kind="ExternalOutput")
        dbg["k8"] = nc.dram_tensor("dbg_k8", [128, CT, N], F8,
                                   kind="ExternalOutput")
        dbg["q8"] = nc.dram_tensor("dbg_q8", [128, CT, QH], F8,
                                   kind="ExternalOutput")
        dbg["vt8"] = nc.dram_tensor("dbg_vt8", [128, MB, C], F8,
                                    kind="ExternalOutput")
        dbg["att"] = nc.dram_tensor("dbg_att", [128, 2, 512], F32,
                                    kind="ExternalOutput")
        dbg["drec"] = nc.dram_tensor("dbg_drec", [1, 512], F32R,
                                     kind="ExternalOutput")
        dbg["exps"] = nc.dram_tensor("dbg_exps", [128, 2, 512], F8,
                                     kind="ExternalOutput")

    handles = dict(X=X, WQ8=WQ8, WK8=WK8, WV8=WV8, WPT16=WPT16, BQ=BQ,
                   BK=BK, PB2=PB2, GAMMA=GAMMA, BETA=BETA, GMASK=GMASK,
                   GBCAST=GBCAST, ONES8=ONES8, ONESROW=ONESROW, EPSC=EPSC,
                   EXPB=EXPB, OUT=OUT)
    try:
        _build_body(nc, dbg, handles)
    except _StopBuild:
        pass
    split_multi_waits(nc)
    return nc


def _build_body(nc, dbg, handles):
    (X, WQ8, WK8, WV8, WPT16, BQ, BK, PB2, GAMMA, BETA, GMASK, GBCAST,
     ONES8, ONESROW, EPSC, EXPB, OUT) = (
        handles[n] for n in (
            "X", "WQ8", "WK8", "WV8", "WPT16", "BQ", "BK", "PB2",
            "GAMMA", "BETA", "GMASK", "GBCAST", "ONES8", "ONESROW",
            "EPSC", "EXPB", "OUT"))
    with tile.TileContext(nc) as tc, nc.allow_low_precision(
            reason="fp8/bf16 attention with fp32 accumulation"):
        with tc.tile_pool(name="big", bufs=1) as big, \
             tc.tile_pool(name="small", bufs=1) as small, \
             tc.tile_pool(name="expp", bufs=3) as expp, \
             tc.tile_pool(name="attp", bufs=2) as attp, \
             tc.tile_pool(name="dbp", bufs=2) as dbp, \
             tc.tile_pool(name="outp", bufs=4) as outp, \
             tc.tile_pool(name="ps_ring", bufs=1, space="PSUM") as ps_ring, \
             tc.tile_pool(name="ps_att", bufs=1, space="PSUM") as ps_attp, \
             tc.tile_pool(name="ps_p", bufs=1, space="PSUM") as ps_pp, \
             tc.tile_pool(name="ps_d", bufs=1, space="PSUM") as ps_dp:

            # ---------------------------------------------- PSUM ring
            ring = ps_ring.tile([128, 4, 512], F32, tag="ring")
            _rctr = [0]

            def rslot():
                i = _rctr[0] % 4
                _rctr[0] += 1
                return ring[:, i, :]

            def rpair():
                if _rctr[0] % 2:
                    _rctr[0] += 1
                i = _rctr[0] % 4
                _rctr[0] += 2
                return ring[:, i:i + 2, :]

            # ------------------------------------------------ load x + weights
            x_sb = [big.tile([128, N], F32, tag=f"x{t}", name=f"x{t}")
                    for t in range(CT)]
            for t in range(CT):
                for hf in range(2):
                    cs = slice(hf * (N // 2), (hf + 1) * (N // 2))
                    nc.sync.dma_start(x_sb[t][:, cs],
                                      X[t * 128:(t + 1) * 128, cs])

            wq8 = small.tile([128, CT, C], F8, tag="wq8")
            wk8 = small.tile([128, CT, C], F8, tag="wk8")
            wv8 = small.tile([128, CT, C], F8, tag="wv8")
            for t_, d_ in ((wq8, WQ8), (wk8, WK8), (wv8, WV8)):
                nc.sync.dma_start(t_[:], d_[:])
            wp16 = [small.tile([128, C], BF16, tag=f"wp{t}", name=f"wp{t}")
                    for t in range(CT)]
            for t in range(CT):
                nc.sync.dma_start(wp16[t][:], WPT16[t * 128:(t + 1) * 128, :])

            bq = small.tile([128, CT], F32, tag="bq")
            bk = small.tile([128, CT], F32, tag="bk")
            pb2 = small.tile([128, CT], F32, tag="pb2")
            gamma = small.tile([128, CT], F32, tag="gamma")
            beta = small.tile([128, CT], F32, tag="beta")
            for t_, d_ in ((bq, BQ), (bk, BK), (pb2, PB2), (gamma, GAMMA),
                           (beta, BETA)):
                nc.sync.dma_start(t_[:], d_[:])
            gmask = small.tile([128, 4], F32R, tag="gmask")
            nc.sync.dma_start(gmask[:], GMASK[:])
            gbcast = small.tile([4, 128], F32R, tag="gbcast")
            nc.sync.dma_start(gbcast[:], GBCAST[:])
            ones8 = small.tile([128, CT, 16], F8, tag="ones8")
            nc.sync.dma_start(ones8[:], ONES8[:])
            onesrow = small.tile([1, 128], F32R, tag="onesrow")
            nc.sync.dma_start(onesrow[:], ONESROW[:])
            epsc = small.tile([128, 1], F32, tag="epsc")
            nc.sync.dma_start(epsc[:], EPSC[:])
            expb = small.tile([128, 1], F32, tag="expb")
            nc.sync.dma_start(expb[:], EXPB[:])

            # ------------------------------------------------ GroupNorm stats
            h8 = big.tile([128, CT, N], F8, tag="h8")
            k8 = big.tile([128, CT, N], F8, tag="k8")
            q8 = big.tile([128, CT, QH], F8, tag="q8")
            vt8 = big.tile([128, MB, C], F8, tag="vt8")

            # Per-half partial stats so each half's reduce/square starts as
            # soon as its DMA lands (overlaps GroupNorm with the x load).
            statsp = [small.tile([128, 4], F32, tag=f"stp{t}", name=f"stp{t}")
                      for t in range(CT)]
            stats = [small.tile([128, 2], F32R, tag=f"st{t}", name=f"st{t}")
                     for t in range(CT)]
            for hf in range(2):
                cs = slice(hf * (N // 2), (hf + 1) * (N // 2))
                for t in range(CT):
                    nc.vector.tensor_reduce(
                        statsp[t][:, hf:hf + 1], x_sb[t][:, cs],
                        axis=mybir.AxisListType.X, op=ALU.add)
                    # Square output is pure scratch (k8 is overwritten
                    # later); the f32 accumulated row-sum is what we keep.
                    nc.scalar.activation(
                        k8[:, t, cs], x_sb[t][:, cs], AF.Square,
                        accum_out=statsp[t][:, 2 + hf:3 + hf])
            for t in range(CT):
                nc.vector.tensor_add(stats[t][:, 0:1], statsp[t][:, 0:1],
                                     statsp[t][:, 1:2])
                nc.vector.tensor_add(stats[t][:, 1:2], statsp[t][:, 2:3],
                                     statsp[t][:, 3:4])

            a_c = []
            d_c = []
            for t in range(CT):
                ps_g = rslot()
                nc.tensor.matmul(ps_g[0:4, 0:2], gmask[:], stats[t][:],
                                 start=True, stop=True)
                gstats = small.tile([4, 2], F32R, tag=f"gst{t}", name=f"gst{t}")
                nc.vector.tensor_copy(gstats[:], ps_g[0:4, 0:2])
                ps_bc = rslot()
                nc.tensor.matmul(ps_bc[:, 0:2], gbcast[:], gstats[:],
                                 start=True, stop=True)
                mean = small.tile([128, 1], F32, tag=f"mean{t}")
                ex2 = small.tile([128, 1], F32, tag=f"ex2{t}")
                nc.scalar.mul(mean[:], ps_bc[:, 0:1], INV_CNT)
                nc.scalar.mul(ex2[:], ps_bc[:, 1:2], INV_CNT)
                var = small.tile([128, 1], F32, tag=f"var{t}")
                # var' = mean^2 - ex2; rstd = exp(-0.5*ln(eps - var'))
                nc.vector.scalar_tensor_tensor(
                    var[:], mean[:], mean[:], ex2[:],
                    op0=ALU.mult, op1=ALU.subtract)
                lnv = small.tile([128, 1], F32, tag=f"lnv{t}")
                nc.scalar.activation(lnv[:], var[:], AF.Ln,
                                     bias=epsc[:], scale=-1.0)
                rstd = small.tile([128, 1], F32, tag=f"rstd{t}")
                nc.scalar.activation(rstd[:], lnv[:], AF.Exp,
                                     bias=0.0, scale=-0.5)
                a_t = small.tile([128, 1], F32, tag=f"a{t}")
                nc.vector.tensor_mul(a_t[:], rstd[:], gamma[:, t:t + 1])
                d_t = small.tile([128, 1], F32, tag=f"d{t}")
                tmp = small.tile([128, 1], F32, tag=f"tmp{t}")
                nc.vector.tensor_mul(tmp[:], mean[:], a_t[:])
                nc.vector.tensor_sub(d_t[:], beta[:, t:t + 1], tmp[:])
                a_c.append(a_t)
                d_c.append(d_t)

            # h8 apply: ct0 on ScalarE, ct1 on DVE (parallel engines; the
            # DVE path measures faster for the fp8 store)
            nc.scalar.activation(h8[:, 0, :], x_sb[0][:], AF.Identity,
                                 bias=d_c[0][:], scale=a_c[0][:])
            nc.vector.tensor_scalar(h8[:, 1, :], x_sb[1][:],
                                    a_c[1][:], d_c[1][:],
                                    op0=ALU.mult, op1=ALU.add)

            # ------------------------------------------------ K / Q (fp8 DR)
            if _DEBUG_STOP < 1:
                raise _StopBuild
            # K[ot*128+m, n] = sum_c wk[ot*128+m, c] h[c, n]; PSUM->SBUF
            # bias-add evictions all run on the DVE: one engine draining
            # back-to-back paces the ring better than alternating engines
            # (each handoff costs two sem round-trips and cools the PE).
            def evict_add(dst, ps, bias_col):
                nc.vector.tensor_scalar_add(dst, ps, bias_col)

            # During production the attention accumulators (ps_att, ps_p)
            # are idle — borrow them as extra pipeline slots so the
            # evictions stream back-to-back instead of lockstepping
            # MM -> evict -> MM on a 2-pair ring.
            _pctr = [0]

            def prod_pair():
                i = _pctr[0] % 3
                _pctr[0] += 1
                if i == 2:
                    pa = ps_attp.tile([128, 2, 512], F32, tag="psatt",
                                      name="prodpa")
                    return pa[:, :, :]
                return rpair()

            _vctr = [0]

            def prod_slot():
                i = _vctr[0] % 5
                _vctr[0] += 1
                if i == 4:
                    pv = ps_pp.tile([128, 512], F32, tag="pp",
                                    name="prodpv")
                    return pv[:, :]
                return rslot()

            for ot in range(CT):
                for cp in range(N // 1024):
                    ps = prod_pair()
                    for j in range(2):
                        ch = 2 * cp + j
                        nc.tensor.matmul(
                            ps[:, j, :],
                            wk8[:, :, ot * 128:(ot + 1) * 128],
                            h8[:, :, ch * 512:(ch + 1) * 512],
                            start=True, stop=True, perf_mode=DR)
                    evict_add(k8[:, ot, cp * 1024:(cp + 1) * 1024], ps,
                              bk[:, ot:ot + 1])
            for ot in range(CT):
                for cp in range(QH // 1024):
                    ps = prod_pair()
                    for j in range(2):
                        ch = 2 * cp + j
                        nc.tensor.matmul(
                            ps[:, j, :],
                            wq8[:, :, ot * 128:(ot + 1) * 128],
                            h8[:, :, ch * 512:(ch + 1) * 512],
                            start=True, stop=True, perf_mode=DR)
                    evict_add(q8[:, ot, cp * 1024:(cp + 1) * 1024], ps,
                              bq[:, ot:ot + 1])

            if _DEBUG_STOP < 2:
                raise _StopBuild
            # ------------------------------------------------ V^T (fp8 DR)
            # VT[u*128+m, vc] = sum_c h[c, u*128+m] wv[vc, c]   (bv -> pb2)
            for up in range(NUP):
                ps = prod_slot()
                for j in range(2):
                    u = 2 * up + j
                    nc.tensor.matmul(
                        ps[:, j * 256:(j + 1) * 256],
                        h8[:, :, u * 128:(u + 1) * 128],
                        wv8[:],
                        start=True, stop=True, perf_mode=DR)
                # V evictions run as a phase-pure Scalar block (DVE still
                # owns the K/Q bias-adds): each engine drains back-to-back.
                nc.scalar.copy(vt8[:, 2 * up:2 * up + 2, :], ps)

            # ------------------------------------------------ attention
            if _DEBUG_STOP < 3:
                raise _StopBuild
            resid = x_sb  # x_sb[ot][:, 0:QH] is this core's residual slice

            # tail state carried from chunk ch-1 into chunk ch's up-loop
            tail = [None]

            def emit_tail(step, ch_prev):
                """step 0: bcast 1/D; 1: proj+final ot0; 2: proj+final ot1.

                The shared ps_p bank serializes bcast -> proj0 -> proj1, but
                each writer only waits on the previous tenant's single DVE
                read, and S^T work is interleaved between steps, so the PE
                stream stays fed."""
                st = tail[0]
                if st is None or st["ch"] != ch_prev:
                    return
                ch = st["ch"]
                qs = slice(ch * 512, (ch + 1) * 512)
                if step == 0:
                    ps_db = ps_pp.tile([128, 512], F32, tag="pp")
                    nc.tensor.matmul(ps_db[:], onesrow[:], st["drec"][:],
                                     start=True, stop=True)
                    db_sb = dbp.tile([128, 512], F32, tag="db")
                    nc.vector.tensor_copy(db_sb[:], ps_db[:])
                    st["db"] = db_sb
                else:
                    ot = step - 1
                    ps_p = ps_pp.tile([128, 512], F32, tag="pp")
                    for ctk in range(CT):
                        nc.tensor.matmul(
                            ps_p[:],
                            wp16[ctk][:, ot * 128:(ot + 1) * 128],
                            st["att"][:, ctk, :],
                            start=(ctk == 0), stop=(ctk == CT - 1))
                    t1 = outp.tile([128, 512], F32, tag="t1")
                    nc.vector.tensor_mul(t1[:], ps_p[:], st["db"][:])
                    o_t = outp.tile([128, 512], F32, tag="o")
                    nc.vector.scalar_tensor_tensor(
                        o_t[:], t1[:], pb2[:, ot:ot + 1],
                        resid[ot][:, qs],
                        op0=ALU.add, op1=ALU.add)
                    nc.sync.dma_start(OUT[ot * 128:(ot + 1) * 128, qs],
                                      o_t[:])
                    if step == 2:
                        tail[0] = None

            # Software-pipelined PE stream: S^T for iteration g+1 is emitted
            # BEFORE PV/D of iteration g, so the PE computes the next scores
            # block while ScalarE runs exp(g) — the exp handoff latency never
            # stalls the (in-order) PE queue.
            def emit_st(g):
                ch, up = divmod(g, NUP)
                qs = slice(ch * 512, (ch + 1) * 512)
                ps_st = rpair()
                for j in range(2):
                    u = 2 * up + j
                    nc.tensor.matmul(
                        ps_st[:, j, :],
                        k8[:, :, u * 128:(u + 1) * 128],
                        q8[:, :, qs],
                        start=True, stop=True, perf_mode=DR)
                return ps_st

            ps_att = None
            ps_d = None
            st_next = emit_st(0) if NCH else None
            for g in range(NCH * NUP):
                ch, up = divmod(g, NUP)
                if up == 0:
                    ps_att = ps_attp.tile([128, 2, 512], F32, tag="psatt")
                    ps_d = ps_dp.tile([1, 512], F32, tag="psd")
                ps_st = st_next
                exps = expp.tile([128, 2, 512], F8, tag="exps")
                nc.scalar.activation(exps[:], ps_st, AF.Exp,
                                     bias=expb[:], scale=SM_SCALE)
                if dbg and g == 0:
                    nc.sync.dma_start(dbg["exps"][:], exps[:])
                if g + 1 < NCH * NUP:
                    st_next = emit_st(g + 1)
                for ct in range(CT):
                    nc.tensor.matmul(
                        ps_att[:, ct, :],
                        vt8[:, 2 * up:2 * up + 2,
                            ct * 128:(ct + 1) * 128],
                        exps[:],
                        start=(up == 0), stop=(up == NUP - 1),
                        perf_mode=DR)
                nc.tensor.matmul(
                    ps_d[:], ones8[:, :, 0:1], exps[:],
                    start=(up == 0), stop=(up == NUP - 1),
                    perf_mode=DR)
                if _DEBUG_STOP >= 5 and up in (3, 5, 7):
                    emit_tail((up - 3) // 2, ch - 1)

                if up != NUP - 1 or _DEBUG_TAIL < 1:
                    continue
                # end of chunk: immediately evict this chunk's accumulators
                att16 = attp.tile([128, 2, 512], BF16, tag="att")
                nc.vector.tensor_copy(att16[:], ps_att[:])
                drec = dbp.tile([1, 512], F32R, tag="drec")
                nc.vector.reciprocal(drec[:], ps_d[:])
                tail[0] = {"ch": ch, "att": att16, "drec": drec}
                if dbg and ch == 0:
                    att32 = attp.tile([128, 2, 512], F32, tag="att32")
                    nc.vector.tensor_copy(att32[:], ps_att[:])
                    nc.sync.dma_start(dbg["att"][:], att32[:])
                    nc.sync.dma_start(dbg["drec"][:], drec[:])
                if _DEBUG_STOP == 4 or (_DEBUG_STOP == 3 and ch == 0):
                    for step in range(min(3, _DEBUG_TAIL - 1)):
                        emit_tail(step, ch)
                if _DEBUG_STOP == 3 and ch == 0:
                    break

            if _DEBUG_STOP >= 5:
                for step in range(3):
                    emit_tail(step, NCH - 1)

            if dbg:
                for name, src in (("h8", h8), ("k8", k8), ("q8", q8),
                                  ("vt8", vt8)):
                    nc.sync.dma_start(dbg[name][:], src[:])


_NC_CACHE = None


def _get_nc():
    global _NC_CACHE
    if _NC_CACHE is None:
        _NC_CACHE = build()
    return _NC_CACHE


def make_in_maps(x, gamma, beta, w_qkv, b_qkv, w_proj, b_proj):
    np_f8 = mybir.dt.np(F8)
    np_bf16 = mybir.dt.np(BF16)
    x = np.asarray(x, np.float32).reshape(B, C, N)
    gamma = np.asarray(gamma, np.float32)
    beta = np.asarray(beta, np.float32)
    w_qkv = np.asarray(w_qkv, np.float32)
    b_qkv = np.asarray(b_qkv, np.float32)
    w_proj = np.asarray(w_proj, np.float32)
    b_proj = np.asarray(b_proj, np.float32)

    wq, wk, wv = w_qkv[0:C], w_qkv[C:2 * C], w_qkv[2 * C:3 * C]
    bq, bk, bv = b_qkv[0:C], b_qkv[C:2 * C], b_qkv[2 * C:3 * C]
    pb2 = (w_proj @ bv + b_proj).astype(np.float32)

    def col2(v):
        return np.ascontiguousarray(v.reshape(CT, 128).T, dtype=np.float32)

    def dr_w(w):
        # [128, CT, C]: out[p, t, o] = w[o, t*128 + p]
        return np.ascontiguousarray(
            w.T.reshape(CT, 128, C).transpose(1, 0, 2)).astype(np_f8)

    gmask = np.zeros((128, 4), np.float32)
    for p in range(128):
        gmask[p, p // 32] = 1.0
    gbcast = np.ascontiguousarray(gmask.T)

    common = {
        "wq8": dr_w(wq),
        "wk8": dr_w(wk),
        "wv8": dr_w(wv),
        "wpt16": np.ascontiguousarray(w_proj.T).astype(np_bf16),
        "bq2": col2(bq),
        "bk2": col2(bk),
        "pb2": col2(pb2),
        "gamma2": col2(gamma),
        "beta2": col2(beta),
        "gmask": gmask,
        "gbcast": gbcast,
        "ones8": np.ones((128, CT, 16), np_f8),
        "ones1x128": np.ones((1, 128), np.float32),
        "epsc": np.full((128, 1), EPS, np.float32),
        "expb": np.full((128, 1), EXP_BIAS, np.float32),
    }

    in_maps = []
    for core in range(NCORES):
        b, half = core // 2, core % 2
        qoff = half * QH
        xc = np.concatenate([x[b][:, qoff:], x[b][:, :qoff]], axis=1)
        m = dict(common)
        m["x"] = np.ascontiguousarray(xc)
        in_maps.append(m)
    return in_maps


def gather_out(results):
    out = np.empty((B, C, N), np.float32)
    for core in range(NCORES):
        b, half = core // 2, core % 2
        qoff = half * QH
        out[b][:, qoff:qoff + QH] = results[core]["out"]
    return out.reshape(B, C, HH, WW)


def kernel(x, gamma, beta, w_qkv, b_qkv, w_proj, b_proj, **run_kwargs):
    nc = _get_nc()
    in_maps = make_in_maps(x, gamma, beta, w_qkv, b_qkv, w_proj, b_proj)
    res = run_bass_kernel_spmd(nc, in_maps, core_ids=list(range(NCORES)),
                               **run_kwargs)
    out = gather_out(res.results)
    kernel.last_results = res
    return out
